# revision 33
# baseline (speedup 1.0000x reference)
"""BSC loss (single label) on 8 Trainium2 NeuronCores — fp8 DoubleRow version.

Reference computation (B=8192, H=256, C=32, T=0.1):
    f   = l2_normalize(features)                      # [B, H]
    sim = f @ f.T / T                                 # [B, B] (never materialized)
    E   = exp(sim) with zeroed diagonal
    class_sum[i, c] = sum_{j: label_j = c} E[i, j]
    counts_excl[i, c] = counts[c] - onehot[i, c]
    denom_i = sum_c where(ce > 0, class_sum / max(ce, 1))
    mean_pos_sim_i = (sum_{j != i, same label} sim[i, j]) / P_i
    loss_i = log(max(denom_i, 1e-30)) - mean_pos_sim_i   (if P_i > 0)
    loss = sum(loss_i) / n_valid

Distribution: each core gets the inputs ROTATED by core*1024 rows and computes
the partial (sum loss_i, n_valid) over rotated rows 0..1023 (its anchor shard).
Pure SPMD; scalar partials summed on the host.

Key structure per core (vs the bf16 baseline):
  - the key side of the similarity is RAW: fT holds fp8(x) transposed
    straight from a stride-2 bf16 view of the fp32 rows (high bytes), so no
    per-key normalize multiply exists; the key-side 1/norm rides in the exp
    activation's per-partition scale AP (rinv10[:, kb]).  Only the 8 anchor
    chunks get explicitly normalized fp8 columns (fT_anch).
  - all big matmuls (similarity slab, class-sum accumulation, positives
    matmul) are fp8e4m3 with MatmulPerfMode.DoubleRow ([128, 2, *] operands,
    K=256 per pass): ~2x PE column rate on hardware.
  - 10/norm computed as exp(-0.5*ln(0.01*n2)) so every ACT call lives in the
    single natural_log_exp_and_others table -> no 1.3us activation-table
    reloads interleaved with the hot exp stream (Sqrt would thrash it).
  - the diagonal (self-pair) is removed by accumulating an extra -64*I fp8
    matmul into the similarity PSUM for key blocks 0..7: the exp argument
    10*r_i*(n_i - 64) <= -23 underflows to zero, so fp8 E never overflows.
  - g (10x class feature sums) uses scaled-onehot weights (DVE-built bf16)
    against the raw bf16 view; its 257th moving column holds n/10 so the same
    accumulation yields class counts to ~0.2%.
  - gpsimd (Pool) only builds the small [128,32] onehots: measured Pool cost
    is ~5x the cost model (generic tensor_scalar [128,256] is ~4us!), so no
    sizable tensor op may live there.
Engine budget (HW-measured): ACT exp stream ~77us but ~90% hidden; the wall
is the DVE chain (sq+evict+ohs+finale) + head/tail, ~150-185us total.
"""

import numpy as np

import bass_rust
import concourse.bass as bass
import concourse.tile as tile
from concourse import mybir
from concourse.bass_utils import run_bass_kernel_spmd

F32 = mybir.dt.float32
BF16 = mybir.dt.bfloat16
FP8 = mybir.dt.float8e4

B = 8192
H = 256
C = 32
N_CORES = 8
SHARD = B // N_CORES          # 1024 anchors per core
N_CHUNKS = B // 128           # 64 row chunks / key blocks
TEMP_INV = 10.0               # 1 / temperature
DR = mybir.MatmulPerfMode.DoubleRow

import os
ABLATE = frozenset(os.environ.get("BASS_ABLATE", "").split(",")) - {""}


class SplitDrainTileContext(tile.TileContext):
    """TileContext that caps sem waits at one per instruction.

    The walrus build in this container rejects instructions carrying more
    than one sync wait ("Too many sync wait commands", e.g. on Drain and
    TensorScalarPtr). Tile freely attaches several waits per instruction, so
    split the surplus onto same-engine nops inserted immediately before the
    instruction (identical semantics: the engine blocks on every wait before
    executing it).
    """

    MAX_DRAIN_WAITS = 1

    def _lower_ordered_insts(self, ordered):
        for insts in ordered.values():
            new_list = []
            for inst in insts:
                si = inst.sync_info
                ws = list(si.on_wait) if si is not None and si.on_wait else []
                if len(ws) > 1:
                    for k, w in enumerate(ws[:-1]):
                        new_list.append(mybir.InstNoOp(
                            name=f"{inst.name}_sw{k}",
                            engine=inst.engine,
                            bass_nofuse=True,
                            sync_info=mybir.SyncInfo(on_wait=[w], on_update=[]),
                        ))
                    inst.sync_info = mybir.SyncInfo(
                        on_wait=[ws[-1]], on_update=list(si.on_update or []))
                new_list.append(inst)
            insts[:] = new_list
        super()._lower_ordered_insts(ordered)

    def _drain_and_barrier(self, tick_clock, wait_clock):
        probe = self.nc.sync.nop()
        wait_clock.add_sem_waits(
            probe.ins, bass_rust.ScopedClock({None: tick_clock.global_clock})
        )
        si = probe.ins.sync_info
        waits = list(si.on_wait) if si is not None and si.on_wait else []
        probe.ins.sync_info = bass_rust.SyncInfo(
            on_wait=waits[: self.MAX_DRAIN_WAITS], on_update=[]
        )
        for i in range(self.MAX_DRAIN_WAITS, len(waits), self.MAX_DRAIN_WAITS):
            n = self.nc.sync.nop()
            n.ins.sync_info = bass_rust.SyncInfo(
                on_wait=waits[i : i + self.MAX_DRAIN_WAITS], on_update=[]
            )
        self.nc.sync.drain()

        self.nc.all_engine_barrier()
        assert self.sems is not None
        popped = self.nc._tile_sem_poison_stack.pop()
        assert popped is self._sem_poison
        self.nc.clear_and_free_semaphores(list(self.sems.allocated().values()))
        self.nc.all_engine_barrier()


def build_program(n_iters: int = 1):
    """Emit the SPMD program. n_iters > 1 wraps the body in a hardware loop
    (identical recompute) for wall-clock timing runs."""
    nc = bass.Bass("TRN2", target_bir_lowering=False, debug=False,
                   num_devices=N_CORES)

    feat = nc.dram_tensor("feat", [B, H], F32, kind="ExternalInput")
    lab = nc.dram_tensor("lab", [128, N_CHUNKS], F32, kind="ExternalInput")
    out = nc.dram_tensor("partials", [1, 2], F32, kind="ExternalOutput")

    with SplitDrainTileContext(nc) as tc:
        if n_iters == 1:
            emit_body(nc, tc, feat, lab, out)
        else:
            hints = (mybir.EngineType.PE, mybir.EngineType.Activation,
                     mybir.EngineType.DVE, mybir.EngineType.SP,
                     mybir.EngineType.Pool)
            with tc.For_i(0, n_iters, 1, hint_engines=hints):
                emit_body(nc, tc, feat, lab, out)
    return nc


def emit_body(nc, tc, feat, lab, out):
    from contextlib import ExitStack

    ACT = mybir.ActivationFunctionType
    OP = mybir.AluOpType
    AX = mybir.AxisListType

    with ExitStack() as ctx:
        ep = ctx.enter_context  # shorthand

        # ---- persistent SBUF ----
        const_pool = ep(tc.tile_pool(name="consts", bufs=1))
        id8 = const_pool.tile([128, 128], FP8)
        from concourse import masks
        masks.make_identity(nc, id8[:])
        id16 = const_pool.tile([128, 128], BF16)
        masks.make_identity(nc, id16[:])
        # dplate[:, q, :]: [128, 512] zeros except -64*I at column offset q*128
        # (keys are RAW rows: diag of ps is x_i . f_i_hat ~ n_i ~ 16; after -64
        # the exp argument 10*r_i*(n_i - 64) <= -23 underflows to zero)
        dplate = const_pool.tile([128, 4, 512], FP8)
        nc.gpsimd.memset(dplate[:], 0.0)
        for q in range(4):
            nc.gpsimd.affine_select(
                out=dplate[:, q, q * 128:(q + 1) * 128],
                in_=dplate[:, q, q * 128:(q + 1) * 128],
                compare_op=OP.not_equal,
                fill=-64.0, base=0, pattern=[[-1, 128]], channel_multiplier=1)
        iota32 = const_pool.tile([128, C], F32)
        nc.gpsimd.iota(iota32[:], pattern=[[1, C]], base=0,
                       channel_multiplier=0,
                       allow_small_or_imprecise_dtypes=True)
        ones32 = const_pool.tile([C, 1], F32)
        nc.gpsimd.memset(ones32[:], 1.0)
        ones16 = const_pool.tile([C, 1], BF16)
        nc.gpsimd.memset(ones16[:], 1.0)

        big_pool = ep(tc.tile_pool(name="big", bufs=1))
        # column H holds n_key/10 so the scaled-onehot g matmul also yields
        # exact-enough class counts: sum (10/n)*(n/10) = count
        x32 = big_pool.tile([128, N_CHUNKS, H + 1], F32)
        fT = big_pool.tile([128, 2, B], FP8)     # fT[p, k, j] = x[j, 128k+p] RAW
        fT_anch = big_pool.tile([128, 2, SHARD], FP8)  # normalized anchor cols
        oh_sb = big_pool.tile([128, N_CHUNKS, C], FP8)
        rinv10 = big_pool.tile([128, N_CHUNKS], F32)   # 10 / ||x_key||
        lab_sb = big_pool.tile([128, N_CHUNKS], F32)
        nc.sync.dma_start(lab_sb[:], lab.ap())

        if ABLATE & {"nosq", "noevict"}:
            nc.gpsimd.memset(rinv10[:], 0.05)
            nc.gpsimd.memset(fT[:], 0.03)
            nc.gpsimd.memset(fT_anch[:], 0.03)
        # truncated-bf16 view of the high bytes of x32 (per chunk, free 256)
        x16v = x32[:].bitcast(mybir.dt.uint16).rearrange(
            "p c (h two) -> p c h two", two=2)[:, :, :, 1:2].bitcast(BF16)

        fr = feat.ap().rearrange("(c p) h -> p c h", p=128)
        for d in range(16):
            nc.sync.dma_start(x32[:, d * 4:(d + 1) * 4, 0:H],
                              fr[:, d * 4:(d + 1) * 4, :])

        # persistent PSUM accumulator for class sums
        cs_pool = ep(tc.tile_pool(name="csacc", bufs=1, space="PSUM"))
        cs_psum = cs_pool.tile([C, SHARD], F32)  # class_sum.T for anchors
        if "noexp" in ABLATE:
            for nb in range(2):
                nc.tensor.matmul(cs_psum[:, nb * 512:(nb + 1) * 512],
                                 id8[:, 0:C], id8[:, 0:512] if False else
                                 dplate[:, 0, :], start=True, stop=True)

        fin = ep(tc.tile_pool(name="fin", bufs=1))

        GRP = 8   # chunks per batched-rsqrt group
        LAG = 7   # hot-loop key block emitted alongside stage-A chunk kb+LAG

        cs_ready = []   # (pair_idx, e2 tile) with both exps emitted
        e2_cell = [None]

        def cs_mms(t, e2):
            for nb in range(2):
                nc.tensor.matmul(
                    cs_psum[:, nb * 512:(nb + 1) * 512],
                    oh_sb[:, 2 * t:2 * t + 2, :],
                    e2[:, :, nb * 512:(nb + 1) * 512],
                    start=(t == 0), stop=(t == N_CHUNKS // 2 - 1),
                    perf_mode=DR)

        with tc.tile_pool(name="simp", bufs=2, space="PSUM") as simpool, \
             tc.tile_pool(name="esb", bufs=3) as epool:

            def hot_iter(kb):
                ps = simpool.tile([128, SHARD], F32, tag="ps")
                for nb in range(2):
                    diag_here = (kb < SHARD // 128) and (kb * 128) // 512 == nb
                    nc.tensor.matmul(
                        ps[:, nb * 512:(nb + 1) * 512],
                        fT[:, :, kb * 128:(kb + 1) * 128],
                        fT_anch[:, :, nb * 512:(nb + 1) * 512],
                        start=True, stop=(not diag_here),
                        perf_mode=DR)
                    if diag_here:
                        # self-pairs: exp argument drops below -23 -> 0
                        nc.tensor.matmul(
                            ps[:, nb * 512:(nb + 1) * 512], id8[:],
                            dplate[:, kb % 4, :],
                            start=False, stop=True)
                if "noexp" in ABLATE:
                    return
                if cs_ready and kb % 2 == 0:
                    cs_mms(*cs_ready.pop(0))
                if kb % 2 == 0:
                    e2 = epool.tile([128, 2, SHARD], FP8, tag="e")
                    e2_cell[0] = e2
                e2 = e2_cell[0]
                nc.scalar.activation(e2[:, kb % 2, :], ps[:], ACT.Exp,
                                     scale=rinv10[:, kb:kb + 1])
                if kb % 2 == 1:
                    cs_ready.append((kb // 2, e2))

            # ---- stage A interleaved with the first hot-loop blocks ----
            with tc.tile_pool(name="gacc", bufs=1, space="PSUM") as g_pool, \
                 tc.tile_pool(name="transp", bufs=1, space="PSUM") as tp_pool, \
                 tc.tile_pool(name="sq", bufs=2) as sqpool, \
                 tc.tile_pool(name="ohs", bufs=2) as ohspool, \
                 tc.tile_pool(name="xba", bufs=2) as xbapool, \
                 tc.tile_pool(name="nrm", bufs=2) as npool:
                g_psum = g_pool.tile([C, H + 1], F32)  # 10*class feat sums|cnt
                for ch in range(N_CHUNKS):
                    g = ch % GRP
                    if g == 0:
                        n2g = npool.tile([128, GRP], F32, tag="n2")
                    sq = sqpool.tile([128, H], F32, tag="sq")
                    if "nosq" in ABLATE and g == 0:
                        nc.vector.memset(n2g[:], 256.0)
                    if "nosq" not in ABLATE:
                        nc.vector.scalar_tensor_tensor(
                            out=sq[:], in0=x32[:, ch, 0:H], scalar=0.0,
                            in1=x32[:, ch, 0:H], op0=OP.bypass, op1=OP.mult,
                            accum_out=n2g[:, g:g + 1])
                    if g == GRP - 1:
                        # 10/norm = exp(-0.5*ln(n2) + ln 10): Exp/Ln table only
                        l2 = npool.tile([128, GRP], F32, tag="l2")
                        nc.scalar.activation(l2[:], n2g[:], ACT.Ln, scale=0.01)
                        nc.scalar.activation(
                            rinv10[:, ch - (GRP - 1):ch + 1], l2[:], ACT.Exp,
                            scale=-0.5)
                        # n/10 = n2 * (10/n) / 100 into column H
                        nc.vector.scalar_tensor_tensor(
                            out=x32[:, ch - (GRP - 1):ch + 1, H:H + 1],
                            in0=n2g[:], scalar=0.01,
                            in1=rinv10[:, ch - (GRP - 1):ch + 1],
                            op0=OP.mult, op1=OP.mult)
                        for j in range(GRP):
                            cj = ch - (GRP - 1) + j
                            nc.gpsimd.tensor_scalar(
                                out=oh_sb[:, cj, :], in0=iota32[:],
                                scalar1=lab_sb[:, cj:cj + 1], scalar2=None,
                                op0=OP.is_equal)
                            # scaled onehot (bf16): (iota==lab) * (10/n_key)
                            ohs = ohspool.tile([128, C], BF16, tag="ohs")
                            nc.vector.tensor_scalar(
                                out=ohs[:], in0=iota32[:],
                                scalar1=lab_sb[:, cj:cj + 1],
                                scalar2=rinv10[:, cj:cj + 1],
                                op0=OP.is_equal, op1=OP.mult)
                            nc.tensor.matmul(
                                g_psum[:], ohs[:],
                                x16v[:, cj:cj + 1, :, :],
                                start=(cj == 0), stop=(cj == N_CHUNKS - 1))
                            if "noevict" in ABLATE:
                                if cj >= LAG:
                                    hot_iter(cj - LAG)
                                continue
                            tpu = tp_pool.tile([128, 512], mybir.dt.uint8,
                                               tag="tp")
                            tp16 = tpu[:].bitcast(BF16)
                            for kc in range(2):
                                nc.tensor.transpose(
                                    tp16[:, kc * 128:(kc + 1) * 128],
                                    x16v[:, cj:cj + 1,
                                         kc * 128:(kc + 1) * 128, :],
                                    id16[:])
                            nc.vector.tensor_copy(
                                fT[:, :, cj * 128:(cj + 1) * 128],
                                tp16.rearrange("p (k n) -> p k n", k=2))
                            if cj < SHARD // 128:
                                # normalized anchor columns (chunks 0..7)
                                xba = xbapool.tile([128, H], FP8, tag="xba")
                                nc.vector.tensor_scalar(
                                    out=xba[:], in0=x32[:, cj, 0:H],
                                    scalar1=rinv10[:, cj:cj + 1], scalar2=0.1,
                                    op0=OP.mult, op1=OP.mult)
                                tpu2 = tp_pool.tile([128, 512],
                                                    mybir.dt.uint8, tag="tp")
                                tpa = tpu2[:].bitcast(FP8).rearrange(
                                    "p (k n two) -> p k n two", k=2, two=2)
                                for kc in range(2):
                                    nc.tensor.transpose(
                                        tpa[:, kc:kc + 1, :, 0:1],
                                        xba[:, kc * 128:(kc + 1) * 128],
                                        id8[:])
                                nc.vector.tensor_copy(
                                    fT_anch[:, :, cj * 128:(cj + 1) * 128],
                                    tpa[:, :, :, 0:1])
                            if cj >= LAG:
                                hot_iter(cj - LAG)

                # ---- pre-tail finale work (independent of class sums) ----
                g8 = fin.tile([C, H], FP8)
                nc.vector.tensor_copy(g8[:], g_psum[:, 0:H])
                counts = fin.tile([C, 1], F32)
                nc.vector.tensor_copy(counts[:], g_psum[:, H:H + 1])

                gT8 = fin.tile([128, 2, C], FP8)
                for kc in range(2):
                    tpgu = tp_pool.tile([128, 512], mybir.dt.uint8, tag="tp")
                    tpg = tpgu[:].bitcast(FP8).rearrange(
                        "p (k n two) -> p k n two", k=2, two=2)
                    nc.tensor.transpose(
                        tpg[:, 0:1, 0:C, 0:1],
                        g8[:, kc * 128:(kc + 1) * 128], id8[0:C, 0:C])
                    nc.vector.tensor_copy(gT8[:, kc:kc + 1, :],
                                          tpg[:, 0:1, 0:C, 0:1])
                ohT = fin.tile([C, SHARD], BF16)
                for bkl in range(SHARD // 128):
                    tpou = tp_pool.tile([128, 512], mybir.dt.uint8, tag="tp")
                    tpo = tpou[:].bitcast(FP8).rearrange(
                        "p (k n two) -> p k n two", k=2, two=2)
                    nc.tensor.transpose(
                        tpo[0:C, 0:1, :, 0:1], oh_sb[:, bkl:bkl + 1, :],
                        id8[:])
                    nc.vector.tensor_copy(
                        ohT[:, bkl * 128:(bkl + 1) * 128],
                        tpo[0:C, 0:1, :, 0:1])

            with tc.tile_pool(name="rpsum", bufs=1, space="PSUM") as rpool:
                r_psum = rpool.tile([C, SHARD], F32)
                for nb in range(2):
                    nc.tensor.matmul(
                        r_psum[:, nb * 512:(nb + 1) * 512], gT8[:],
                        fT_anch[:, :, nb * 512:(nb + 1) * 512],
                        start=True, stop=True, perf_mode=DR)

                # counts_excl, masks, positives numerator / denominator
                ce = fin.tile([C, SHARD], BF16)
                nc.vector.tensor_scalar(
                    out=ce[:], in0=ohT[:], scalar1=counts[:], scalar2=-1.0,
                    op0=OP.subtract, op1=OP.mult)
                mask = fin.tile([C, SHARD], BF16)
                nc.vector.tensor_single_scalar(mask[:], ce[:], 0.5, OP.is_gt)
                ce1 = fin.tile([C, SHARD], BF16)
                nc.vector.tensor_single_scalar(ce1[:], ce[:], 1.0, OP.max)
                rce = fin.tile([C, SHARD], BF16)
                with nc.allow_low_precision(
                        reason="1/count at bf16: 0.4% on denom terms, "
                               "well under the 2e-2 gate"):
                    nc.vector.reciprocal(rce[:], ce1[:])
                nc.vector.tensor_tensor(rce[:], rce[:], mask[:], OP.mult)

                # r_psum rows hold 10*sum_{j in c} cos(i,j); self term ~ 10
                pnum = fin.tile([C, SHARD], BF16)
                nc.vector.scalar_tensor_tensor(
                    out=pnum[:], in0=r_psum[:], scalar=10.0, in1=ohT[:],
                    op0=OP.subtract, op1=OP.mult)
                pden = fin.tile([C, SHARD], BF16)
                nc.vector.tensor_tensor(pden[:], ohT[:], ce[:], OP.mult)

            with tc.tile_pool(name="rows_pre", bufs=1, space="PSUM") as rowp:
                prow = fin.tile([1, SHARD], F32)
                posrow = fin.tile([1, SHARD], F32)
                for src, dst in ((pden, prow), (pnum, posrow)):
                    row = rowp.tile([1, SHARD], F32, tag="row")
                    for nb in range(2):
                        nc.tensor.matmul(
                            row[:, nb * 512:(nb + 1) * 512], ones16[:],
                            src[:, nb * 512:(nb + 1) * 512],
                            start=True, stop=True)
                    nc.vector.tensor_copy(dst[:], row[:])

                valid = fin.tile([1, SHARD], F32)
                nc.vector.tensor_single_scalar(
                    valid[:], prow[:], 0.5, OP.is_gt)
                nc.vector.tensor_single_scalar(prow[:], prow[:], 1.0, OP.max)
                rp = fin.tile([1, SHARD], F32)
                nc.vector.reciprocal(rp[:], prow[:])
                mp = fin.tile([1, SHARD], F32)
                nc.vector.scalar_tensor_tensor(
                    out=mp[:], in0=posrow[:], scalar=1.0, in1=rp[:],
                    op0=OP.mult, op1=OP.mult)

            # ---- remaining hot-loop blocks ----
            for kb in range(N_CHUNKS - LAG, N_CHUNKS):
                hot_iter(kb)
            while cs_ready:
                cs_mms(*cs_ready.pop(0))

        # ---- post-hot finale: denominator path and output ----
        with tc.tile_pool(name="rows_post", bufs=1, space="PSUM") as rowpool:
            terms = fin.tile([C, SHARD], BF16)
            nc.vector.tensor_tensor(terms[:], cs_psum[:], rce[:], OP.mult)
            logd = fin.tile([1, SHARD], F32)
            row = rowpool.tile([1, SHARD], F32)
            for nb in range(2):
                nc.tensor.matmul(
                    row[:, nb * 512:(nb + 1) * 512], ones16[:],
                    terms[:, nb * 512:(nb + 1) * 512],
                    start=True, stop=True)
            nc.vector.tensor_single_scalar(logd[:], row[:], 1e-30, OP.max)
            nc.scalar.activation(logd[:], logd[:], ACT.Ln)

            li = fin.tile([1, SHARD], F32)
            nc.vector.tensor_tensor(li[:], logd[:], mp[:], OP.subtract)
            nc.vector.tensor_tensor(li[:], li[:], valid[:], OP.mult)

            res = fin.tile([1, 2], F32)
            nc.vector.tensor_reduce(res[:, 0:1], li[:], axis=AX.X, op=OP.add)
            nc.vector.tensor_reduce(res[:, 1:2], valid[:], axis=AX.X, op=OP.add)
            nc.sync.dma_start(out.ap(), res[:])


_PROGRAM_CACHE = {}


def get_program(n_iters: int = 1):
    if n_iters not in _PROGRAM_CACHE:
        _PROGRAM_CACHE[n_iters] = build_program(n_iters)
    return _PROGRAM_CACHE[n_iters]


def make_in_maps(features: np.ndarray, labels: np.ndarray):
    features = np.ascontiguousarray(np.asarray(features, dtype=np.float32))
    labels_f = np.asarray(labels).astype(np.float32)
    in_maps = []
    for c in range(N_CORES):
        fr = np.roll(features, -c * SHARD, axis=0)
        lr = np.roll(labels_f, -c * SHARD)
        in_maps.append({
            "feat": np.ascontiguousarray(fr),
            "lab": np.ascontiguousarray(lr.reshape(N_CHUNKS, 128).T),
        })
    return in_maps


def kernel(features, labels):
    nc = get_program(1)
    in_maps = make_in_maps(features, labels)
    res = run_bass_kernel_spmd(nc, in_maps, list(range(N_CORES)))
    loss_sum = np.float32(0.0)
    n_valid = np.float32(0.0)
    for c in range(N_CORES):
        p = res.results[c]["partials"]
        loss_sum += np.float32(p[0, 0])
        n_valid += np.float32(p[0, 1])
    if n_valid > 0:
        loss = loss_sum / np.float32(max(n_valid, 1.0))
    else:
        loss = np.float32(0.0)
    return np.array(loss, dtype=np.float32)


# revision 38
# speedup vs baseline: 1.1644x; 1.1644x over previous
"""BSC loss (single label) on 8 Trainium2 NeuronCores — fp8 DoubleRow version.

Reference computation (B=8192, H=256, C=32, T=0.1):
    f   = l2_normalize(features)                      # [B, H]
    sim = f @ f.T / T                                 # [B, B] (never materialized)
    E   = exp(sim) with zeroed diagonal
    class_sum[i, c] = sum_{j: label_j = c} E[i, j]
    counts_excl[i, c] = counts[c] - onehot[i, c]
    denom_i = sum_c where(ce > 0, class_sum / max(ce, 1))
    mean_pos_sim_i = (sum_{j != i, same label} sim[i, j]) / P_i
    loss_i = log(max(denom_i, 1e-30)) - mean_pos_sim_i   (if P_i > 0)
    loss = sum(loss_i) / n_valid

Distribution: each core gets the inputs ROTATED by core*1024 rows and computes
the partial (sum loss_i, n_valid) over rotated rows 0..1023 (its anchor shard).
Pure SPMD; scalar partials summed on the host.

Key structure per core (vs the bf16 baseline):
  - the key side of the similarity is RAW: fT holds fp8(x) transposed
    straight from a stride-2 bf16 view of the fp32 rows (high bytes), so no
    per-key normalize multiply exists; the key-side 1/norm rides in the exp
    activation's per-partition scale AP (rinv10[:, kb]).  Only the 8 anchor
    chunks get explicitly normalized fp8 columns (fT_anch).
  - all big matmuls (similarity slab, class-sum accumulation, positives
    matmul) are fp8e4m3 with MatmulPerfMode.DoubleRow ([128, 2, *] operands,
    K=256 per pass): ~2x PE column rate on hardware.
  - 10/norm computed as exp(-0.5*ln(0.01*n2)) so every ACT call lives in the
    single natural_log_exp_and_others table -> no 1.3us activation-table
    reloads interleaved with the hot exp stream (Sqrt would thrash it).
  - the diagonal (self-pair) is removed by accumulating an extra -64*I fp8
    matmul into the similarity PSUM for key blocks 0..7: the exp argument
    10*r_i*(n_i - 64) <= -23 underflows to zero, so fp8 E never overflows.
  - g (10x class feature sums) uses scaled-onehot weights (DVE-built bf16)
    against the raw bf16 view; its 257th moving column holds n/10 so the same
    accumulation yields class counts to ~0.2%.
  - gpsimd (Pool) only builds the small [128,32] onehots: measured Pool cost
    is ~5x the cost model (generic tensor_scalar [128,256] is ~4us!), so no
    sizable tensor op may live there.
Engine budget (HW-measured): ACT exp stream ~77us but ~90% hidden; the wall
is the DVE chain (sq+evict+ohs+finale) + head/tail, ~150-185us total.
"""

import numpy as np

import bass_rust
import concourse.bass as bass
import concourse.tile as tile
from concourse import mybir
from concourse.bass_utils import run_bass_kernel_spmd

F32 = mybir.dt.float32
BF16 = mybir.dt.bfloat16
FP8 = mybir.dt.float8e4

B = 8192
H = 256
C = 32
N_CORES = 8
SHARD = B // N_CORES          # 1024 anchors per core
N_CHUNKS = B // 128           # 64 row chunks / key blocks
TEMP_INV = 10.0               # 1 / temperature
DR = mybir.MatmulPerfMode.DoubleRow

import os
ABLATE = frozenset(os.environ.get("BASS_ABLATE", "").split(",")) - {""}


class SplitDrainTileContext(tile.TileContext):
    """TileContext that caps sem waits at one per instruction.

    The walrus build in this container rejects instructions carrying more
    than one sync wait ("Too many sync wait commands", e.g. on Drain and
    TensorScalarPtr). Tile freely attaches several waits per instruction, so
    split the surplus onto same-engine nops inserted immediately before the
    instruction (identical semantics: the engine blocks on every wait before
    executing it).
    """

    MAX_DRAIN_WAITS = 1

    def _lower_ordered_insts(self, ordered):
        for insts in ordered.values():
            new_list = []
            for inst in insts:
                si = inst.sync_info
                ws = list(si.on_wait) if si is not None and si.on_wait else []
                if len(ws) > 1:
                    for k, w in enumerate(ws[:-1]):
                        new_list.append(mybir.InstNoOp(
                            name=f"{inst.name}_sw{k}",
                            engine=inst.engine,
                            bass_nofuse=True,
                            sync_info=mybir.SyncInfo(on_wait=[w], on_update=[]),
                        ))
                    inst.sync_info = mybir.SyncInfo(
                        on_wait=[ws[-1]], on_update=list(si.on_update or []))
                new_list.append(inst)
            insts[:] = new_list
        super()._lower_ordered_insts(ordered)

    def _drain_and_barrier(self, tick_clock, wait_clock):
        probe = self.nc.sync.nop()
        wait_clock.add_sem_waits(
            probe.ins, bass_rust.ScopedClock({None: tick_clock.global_clock})
        )
        si = probe.ins.sync_info
        waits = list(si.on_wait) if si is not None and si.on_wait else []
        probe.ins.sync_info = bass_rust.SyncInfo(
            on_wait=waits[: self.MAX_DRAIN_WAITS], on_update=[]
        )
        for i in range(self.MAX_DRAIN_WAITS, len(waits), self.MAX_DRAIN_WAITS):
            n = self.nc.sync.nop()
            n.ins.sync_info = bass_rust.SyncInfo(
                on_wait=waits[i : i + self.MAX_DRAIN_WAITS], on_update=[]
            )
        self.nc.sync.drain()

        self.nc.all_engine_barrier()
        assert self.sems is not None
        popped = self.nc._tile_sem_poison_stack.pop()
        assert popped is self._sem_poison
        self.nc.clear_and_free_semaphores(list(self.sems.allocated().values()))
        self.nc.all_engine_barrier()


def build_program(n_iters: int = 1):
    """Emit the SPMD program. n_iters > 1 wraps the body in a hardware loop
    (identical recompute) for wall-clock timing runs."""
    nc = bass.Bass("TRN2", target_bir_lowering=False, debug=False,
                   num_devices=N_CORES)

    feat = nc.dram_tensor("feat", [B, H], F32, kind="ExternalInput")
    lab = nc.dram_tensor("lab", [128, N_CHUNKS], F32, kind="ExternalInput")
    out = nc.dram_tensor("partials", [1, 2], F32, kind="ExternalOutput")

    with SplitDrainTileContext(nc) as tc:
        if n_iters == 1:
            emit_body(nc, tc, feat, lab, out)
        else:
            hints = (mybir.EngineType.PE, mybir.EngineType.Activation,
                     mybir.EngineType.DVE, mybir.EngineType.SP,
                     mybir.EngineType.Pool)
            with tc.For_i(0, n_iters, 1, hint_engines=hints):
                emit_body(nc, tc, feat, lab, out)
    return nc


def emit_body(nc, tc, feat, lab, out):
    from contextlib import ExitStack

    ACT = mybir.ActivationFunctionType
    OP = mybir.AluOpType
    AX = mybir.AxisListType

    with ExitStack() as ctx:
        ep = ctx.enter_context  # shorthand

        # ---- persistent SBUF ----
        const_pool = ep(tc.tile_pool(name="consts", bufs=1))
        id8 = const_pool.tile([128, 128], FP8)
        from concourse import masks
        masks.make_identity(nc, id8[:])
        id16 = const_pool.tile([128, 128], BF16)
        masks.make_identity(nc, id16[:])
        # dplate[:, q, :]: [128, 512] zeros except -64*I at column offset q*128
        # (keys are RAW rows: diag of ps is x_i . f_i_hat ~ n_i ~ 16; after -64
        # the exp argument 10*r_i*(n_i - 64) <= -23 underflows to zero)
        dplate = const_pool.tile([128, 4, 512], FP8)
        nc.gpsimd.memset(dplate[:], 0.0)
        for q in range(4):
            nc.gpsimd.affine_select(
                out=dplate[:, q, q * 128:(q + 1) * 128],
                in_=dplate[:, q, q * 128:(q + 1) * 128],
                compare_op=OP.not_equal,
                fill=-64.0, base=0, pattern=[[-1, 128]], channel_multiplier=1)
        iota32 = const_pool.tile([128, C], F32)
        nc.gpsimd.iota(iota32[:], pattern=[[1, C]], base=0,
                       channel_multiplier=0,
                       allow_small_or_imprecise_dtypes=True)
        ones32 = const_pool.tile([C, 1], F32)
        nc.gpsimd.memset(ones32[:], 1.0)
        ones16 = const_pool.tile([C, 1], BF16)
        nc.gpsimd.memset(ones16[:], 1.0)

        big_pool = ep(tc.tile_pool(name="big", bufs=1))
        # column H holds n_key/10 so the scaled-onehot g matmul also yields
        # exact-enough class counts: sum (10/n)*(n/10) = count
        x32 = big_pool.tile([128, N_CHUNKS, H + 1], F32)
        fT = big_pool.tile([128, 2, B], FP8)     # fT[p, k, j] = x[j, 128k+p] RAW
        fT_anch = big_pool.tile([128, 2, SHARD], FP8)  # normalized anchor cols
        oh_sb = big_pool.tile([128, N_CHUNKS, C], FP8)
        rinv10 = big_pool.tile([128, N_CHUNKS], F32)   # 10 / ||x_key||
        lab_sb = big_pool.tile([128, N_CHUNKS], F32)
        nc.sync.dma_start(lab_sb[:], lab.ap())

        if ABLATE & {"nosq", "noevict"}:
            nc.gpsimd.memset(rinv10[:], 0.05)
            nc.gpsimd.memset(fT[:], 0.03)
            nc.gpsimd.memset(fT_anch[:], 0.03)
        # truncated-bf16 view of the high bytes of x32 (per chunk, free 256)
        x16v = x32[:].bitcast(mybir.dt.uint16).rearrange(
            "p c (h two) -> p c h two", two=2)[:, :, :, 1:2].bitcast(BF16)

        fr = feat.ap().rearrange("(c p) h -> p c h", p=128)
        for d in range(16):
            nc.sync.dma_start(x32[:, d * 4:(d + 1) * 4, 0:H],
                              fr[:, d * 4:(d + 1) * 4, :])

        # persistent PSUM accumulator for class sums
        cs_pool = ep(tc.tile_pool(name="csacc", bufs=1, space="PSUM"))
        cs_psum = cs_pool.tile([C, SHARD], F32)  # class_sum.T for anchors
        if "noexp" in ABLATE:
            for nb in range(2):
                nc.tensor.matmul(cs_psum[:, nb * 512:(nb + 1) * 512],
                                 id8[:, 0:C], id8[:, 0:512] if False else
                                 dplate[:, 0, :], start=True, stop=True)

        fin = ep(tc.tile_pool(name="fin", bufs=1))

        GRP = 8   # chunks per batched-rsqrt group
        LAG = 7   # hot-loop key block emitted alongside stage-A chunk kb+LAG

        cs_ready = []   # (pair_idx, e2 tile) with both exps emitted
        e2_cell = [None]

        def cs_mms(t, e2):
            for nb in range(2):
                nc.tensor.matmul(
                    cs_psum[:, nb * 512:(nb + 1) * 512],
                    oh_sb[:, 2 * t:2 * t + 2, :],
                    e2[:, :, nb * 512:(nb + 1) * 512],
                    start=(t == 0), stop=(t == N_CHUNKS // 2 - 1),
                    perf_mode=DR)

        with tc.tile_pool(name="simp", bufs=2, space="PSUM") as simpool, \
             tc.tile_pool(name="esb", bufs=3) as epool:

            def hot_iter(kb):
                ps = simpool.tile([128, SHARD], F32, tag="ps")
                for nb in range(2):
                    diag_here = (kb < SHARD // 128) and (kb * 128) // 512 == nb
                    nc.tensor.matmul(
                        ps[:, nb * 512:(nb + 1) * 512],
                        fT[:, :, kb * 128:(kb + 1) * 128],
                        fT_anch[:, :, nb * 512:(nb + 1) * 512],
                        start=True, stop=(not diag_here),
                        perf_mode=DR)
                    if diag_here:
                        # self-pairs: exp argument drops below -23 -> 0
                        nc.tensor.matmul(
                            ps[:, nb * 512:(nb + 1) * 512], id8[:],
                            dplate[:, kb % 4, :],
                            start=False, stop=True)
                if "noexp" in ABLATE:
                    return
                if cs_ready and kb % 2 == 0:
                    cs_mms(*cs_ready.pop(0))
                if kb % 2 == 0:
                    e2 = epool.tile([128, 2, SHARD], FP8, tag="e")
                    e2_cell[0] = e2
                e2 = e2_cell[0]
                nc.scalar.activation(e2[:, kb % 2, :], ps[:], ACT.Exp,
                                     scale=rinv10[:, kb:kb + 1])
                if kb % 2 == 1:
                    cs_ready.append((kb // 2, e2))

            # ---- stage A interleaved with the first hot-loop blocks ----
            with tc.tile_pool(name="gacc", bufs=1, space="PSUM") as g_pool, \
                 tc.tile_pool(name="transp", bufs=1, space="PSUM") as tp_pool, \
                 tc.tile_pool(name="sq", bufs=2) as sqpool, \
                 tc.tile_pool(name="ohs", bufs=2) as ohspool, \
                 tc.tile_pool(name="xba", bufs=2) as xbapool, \
                 tc.tile_pool(name="nrm", bufs=2) as npool:
                g_psum = g_pool.tile([C, H + 1], F32)  # 10*class feat sums|cnt

                def do_chunk(cj):
                    nc.gpsimd.tensor_scalar(
                        out=oh_sb[:, cj, :], in0=iota32[:],
                        scalar1=lab_sb[:, cj:cj + 1], scalar2=None,
                        op0=OP.is_equal)
                    # scaled onehot (bf16): (iota==lab) * (10/n_key)
                    ohs = ohspool.tile([128, C], BF16, tag="ohs")
                    nc.vector.tensor_scalar(
                        out=ohs[:], in0=iota32[:],
                        scalar1=lab_sb[:, cj:cj + 1],
                        scalar2=rinv10[:, cj:cj + 1],
                        op0=OP.is_equal, op1=OP.mult)
                    nc.tensor.matmul(
                        g_psum[:], ohs[:],
                        x16v[:, cj:cj + 1, :, :],
                        start=(cj == 0), stop=(cj == N_CHUNKS - 1))
                    if "noevict" not in ABLATE:
                        tpu = tp_pool.tile([128, 512], mybir.dt.uint8,
                                           tag="tp")
                        tp16 = tpu[:].bitcast(BF16)
                        for kc in range(2):
                            nc.tensor.transpose(
                                tp16[:, kc * 128:(kc + 1) * 128],
                                x16v[:, cj:cj + 1,
                                     kc * 128:(kc + 1) * 128, :],
                                id16[:])
                        nc.vector.tensor_copy(
                            fT[:, :, cj * 128:(cj + 1) * 128],
                            tp16.rearrange("p (k n) -> p k n", k=2))
                        if cj < SHARD // 128:
                            # normalized anchor columns (chunks 0..7)
                            xba = xbapool.tile([128, H], FP8, tag="xba")
                            nc.vector.tensor_scalar(
                                out=xba[:], in0=x32[:, cj, 0:H],
                                scalar1=rinv10[:, cj:cj + 1], scalar2=0.1,
                                op0=OP.mult, op1=OP.mult)
                            tpu2 = tp_pool.tile([128, 512],
                                                mybir.dt.uint8, tag="tp")
                            tpa = tpu2[:].bitcast(FP8).rearrange(
                                "p (k n two) -> p k n two", k=2, two=2)
                            for kc in range(2):
                                nc.tensor.transpose(
                                    tpa[:, kc:kc + 1, :, 0:1],
                                    xba[:, kc * 128:(kc + 1) * 128],
                                    id8[:])
                            nc.vector.tensor_copy(
                                fT_anch[:, :, cj * 128:(cj + 1) * 128],
                                tpa[:, :, :, 0:1])
                    if cj == LAG:
                        hot_iter(0)
                        hot_iter(1)
                    elif LAG < cj < 2 * LAG:
                        hot_iter(2 * (cj - LAG))
                        hot_iter(2 * (cj - LAG) + 1)
                    elif cj >= 2 * LAG:
                        hot_iter(cj)

                for ch in range(N_CHUNKS):
                    g = ch % GRP
                    if g == 0:
                        n2g = npool.tile([128, GRP], F32, tag="n2")
                    sq = sqpool.tile([128, H], BF16, tag="sq")
                    if "nosq" in ABLATE and g == 0:
                        nc.vector.memset(n2g[:], 256.0)
                    if "nosq" not in ABLATE:
                        # n2 from the truncated-bf16 view (consistent with the
                        # fp8 fT quantization); bf16 in/out halves DVE traffic
                        nc.vector.scalar_tensor_tensor(
                            out=sq[:], in0=x16v[:, ch:ch + 1, 0:H, :],
                            scalar=0.0,
                            in1=x16v[:, ch:ch + 1, 0:H, :],
                            op0=OP.bypass, op1=OP.mult,
                            accum_out=n2g[:, g:g + 1])
                    if ch < GRP:
                        # group 0: per-chunk rsqrt so the anchor-column chain
                        # (gating the first hot block) starts ASAP
                        l2c = npool.tile([128, 1], F32, tag="l2c")
                        nc.scalar.activation(l2c[:], n2g[:, g:g + 1], ACT.Ln,
                                             scale=0.01)
                        nc.scalar.activation(rinv10[:, ch:ch + 1], l2c[:],
                                             ACT.Exp, scale=-0.5)
                        nc.vector.scalar_tensor_tensor(
                            out=x32[:, ch:ch + 1, H:H + 1],
                            in0=n2g[:, g:g + 1], scalar=0.01,
                            in1=rinv10[:, ch:ch + 1],
                            op0=OP.mult, op1=OP.mult)
                        do_chunk(ch)
                    elif g == GRP - 1:
                        # 10/norm = exp(-0.5*ln(n2) + ln 10): Exp/Ln table only
                        l2 = npool.tile([128, GRP], F32, tag="l2")
                        nc.scalar.activation(l2[:], n2g[:], ACT.Ln, scale=0.01)
                        nc.scalar.activation(
                            rinv10[:, ch - (GRP - 1):ch + 1], l2[:], ACT.Exp,
                            scale=-0.5)
                        # n/10 = n2 * (10/n) / 100 into column H
                        nc.vector.scalar_tensor_tensor(
                            out=x32[:, ch - (GRP - 1):ch + 1, H:H + 1],
                            in0=n2g[:], scalar=0.01,
                            in1=rinv10[:, ch - (GRP - 1):ch + 1],
                            op0=OP.mult, op1=OP.mult)
                        for j in range(GRP):
                            do_chunk(ch - (GRP - 1) + j)

                # ---- pre-tail finale work (independent of class sums) ----
                g8 = fin.tile([C, H], FP8)
                nc.vector.tensor_copy(g8[:], g_psum[:, 0:H])
                counts = fin.tile([C, 1], F32)
                nc.vector.tensor_copy(counts[:], g_psum[:, H:H + 1])

                gT8 = fin.tile([128, 2, C], FP8)
                for kc in range(2):
                    tpgu = tp_pool.tile([128, 512], mybir.dt.uint8, tag="tp")
                    tpg = tpgu[:].bitcast(FP8).rearrange(
                        "p (k n two) -> p k n two", k=2, two=2)
                    nc.tensor.transpose(
                        tpg[:, 0:1, 0:C, 0:1],
                        g8[:, kc * 128:(kc + 1) * 128], id8[0:C, 0:C])
                    nc.vector.tensor_copy(gT8[:, kc:kc + 1, :],
                                          tpg[:, 0:1, 0:C, 0:1])
                ohT = fin.tile([C, SHARD], BF16)
                for bkl in range(SHARD // 128):
                    tpou = tp_pool.tile([128, 512], mybir.dt.uint8, tag="tp")
                    tpo = tpou[:].bitcast(FP8).rearrange(
                        "p (k n two) -> p k n two", k=2, two=2)
                    nc.tensor.transpose(
                        tpo[0:C, 0:1, :, 0:1], oh_sb[:, bkl:bkl + 1, :],
                        id8[:])
                    nc.vector.tensor_copy(
                        ohT[:, bkl * 128:(bkl + 1) * 128],
                        tpo[0:C, 0:1, :, 0:1])

            with tc.tile_pool(name="rpsum", bufs=1, space="PSUM") as rpool:
                r_psum = rpool.tile([C, SHARD], F32)
                for nb in range(2):
                    nc.tensor.matmul(
                        r_psum[:, nb * 512:(nb + 1) * 512], gT8[:],
                        fT_anch[:, :, nb * 512:(nb + 1) * 512],
                        start=True, stop=True, perf_mode=DR)

                # counts_excl, masks, positives numerator / denominator
                ce = fin.tile([C, SHARD], BF16)
                nc.vector.tensor_scalar(
                    out=ce[:], in0=ohT[:], scalar1=counts[:], scalar2=-1.0,
                    op0=OP.subtract, op1=OP.mult)
                mask = fin.tile([C, SHARD], BF16)
                nc.vector.tensor_single_scalar(mask[:], ce[:], 0.5, OP.is_gt)
                ce1 = fin.tile([C, SHARD], BF16)
                nc.vector.tensor_single_scalar(ce1[:], ce[:], 1.0, OP.max)
                rce = fin.tile([C, SHARD], BF16)
                with nc.allow_low_precision(
                        reason="1/count at bf16: 0.4% on denom terms, "
                               "well under the 2e-2 gate"):
                    nc.vector.reciprocal(rce[:], ce1[:])
                nc.vector.tensor_tensor(rce[:], rce[:], mask[:], OP.mult)

                # r_psum rows hold 10*sum_{j in c} cos(i,j); self term ~ 10
                pnum = fin.tile([C, SHARD], BF16)
                nc.vector.scalar_tensor_tensor(
                    out=pnum[:], in0=r_psum[:], scalar=10.0, in1=ohT[:],
                    op0=OP.subtract, op1=OP.mult)
                pden = fin.tile([C, SHARD], BF16)
                nc.vector.tensor_tensor(pden[:], ohT[:], ce[:], OP.mult)

            with tc.tile_pool(name="rows_pre", bufs=1, space="PSUM") as rowp:
                prow = fin.tile([1, SHARD], F32)
                posrow = fin.tile([1, SHARD], F32)
                for src, dst in ((pden, prow), (pnum, posrow)):
                    row = rowp.tile([1, SHARD], F32, tag="row")
                    for nb in range(2):
                        nc.tensor.matmul(
                            row[:, nb * 512:(nb + 1) * 512], ones16[:],
                            src[:, nb * 512:(nb + 1) * 512],
                            start=True, stop=True)
                    nc.vector.tensor_copy(dst[:], row[:])

                valid = fin.tile([1, SHARD], F32)
                nc.vector.tensor_single_scalar(
                    valid[:], prow[:], 0.5, OP.is_gt)
                nc.vector.tensor_single_scalar(prow[:], prow[:], 1.0, OP.max)
                rp = fin.tile([1, SHARD], F32)
                nc.vector.reciprocal(rp[:], prow[:])
                mp = fin.tile([1, SHARD], F32)
                nc.vector.scalar_tensor_tensor(
                    out=mp[:], in0=posrow[:], scalar=1.0, in1=rp[:],
                    op0=OP.mult, op1=OP.mult)

            # all hot blocks were emitted inside stage A; flush class sums
            while cs_ready:
                cs_mms(*cs_ready.pop(0))

        # ---- post-hot finale: denominator path and output ----
        with tc.tile_pool(name="rows_post", bufs=1, space="PSUM") as rowpool:
            terms = fin.tile([C, SHARD], BF16)
            nc.vector.tensor_tensor(terms[:], cs_psum[:], rce[:], OP.mult)
            logd = fin.tile([1, SHARD], F32)
            row = rowpool.tile([1, SHARD], F32)
            for nb in range(2):
                nc.tensor.matmul(
                    row[:, nb * 512:(nb + 1) * 512], ones16[:],
                    terms[:, nb * 512:(nb + 1) * 512],
                    start=True, stop=True)
            nc.vector.tensor_single_scalar(logd[:], row[:], 1e-30, OP.max)
            nc.scalar.activation(logd[:], logd[:], ACT.Ln)

            li = fin.tile([1, SHARD], F32)
            nc.vector.tensor_tensor(li[:], logd[:], mp[:], OP.subtract)
            nc.vector.tensor_tensor(li[:], li[:], valid[:], OP.mult)

            res = fin.tile([1, 2], F32)
            nc.vector.tensor_reduce(res[:, 0:1], li[:], axis=AX.X, op=OP.add)
            nc.vector.tensor_reduce(res[:, 1:2], valid[:], axis=AX.X, op=OP.add)
            nc.sync.dma_start(out.ap(), res[:])


_PROGRAM_CACHE = {}


def get_program(n_iters: int = 1):
    if n_iters not in _PROGRAM_CACHE:
        _PROGRAM_CACHE[n_iters] = build_program(n_iters)
    return _PROGRAM_CACHE[n_iters]


def make_in_maps(features: np.ndarray, labels: np.ndarray):
    features = np.ascontiguousarray(np.asarray(features, dtype=np.float32))
    labels_f = np.asarray(labels).astype(np.float32)
    in_maps = []
    for c in range(N_CORES):
        fr = np.roll(features, -c * SHARD, axis=0)
        lr = np.roll(labels_f, -c * SHARD)
        in_maps.append({
            "feat": np.ascontiguousarray(fr),
            "lab": np.ascontiguousarray(lr.reshape(N_CHUNKS, 128).T),
        })
    return in_maps


def kernel(features, labels):
    nc = get_program(1)
    in_maps = make_in_maps(features, labels)
    res = run_bass_kernel_spmd(nc, in_maps, list(range(N_CORES)))
    loss_sum = np.float32(0.0)
    n_valid = np.float32(0.0)
    for c in range(N_CORES):
        p = res.results[c]["partials"]
        loss_sum += np.float32(p[0, 0])
        n_valid += np.float32(p[0, 1])
    if n_valid > 0:
        loss = loss_sum / np.float32(max(n_valid, 1.0))
    else:
        loss = np.float32(0.0)
    return np.array(loss, dtype=np.float32)


# revision 39
# speedup vs baseline: 1.2536x; 1.0767x over previous
"""BSC loss (single label) on 8 Trainium2 NeuronCores — fp8 DoubleRow version.

Reference computation (B=8192, H=256, C=32, T=0.1):
    f   = l2_normalize(features)                      # [B, H]
    sim = f @ f.T / T                                 # [B, B] (never materialized)
    E   = exp(sim) with zeroed diagonal
    class_sum[i, c] = sum_{j: label_j = c} E[i, j]
    counts_excl[i, c] = counts[c] - onehot[i, c]
    denom_i = sum_c where(ce > 0, class_sum / max(ce, 1))
    mean_pos_sim_i = (sum_{j != i, same label} sim[i, j]) / P_i
    loss_i = log(max(denom_i, 1e-30)) - mean_pos_sim_i   (if P_i > 0)
    loss = sum(loss_i) / n_valid

Distribution: each core gets the inputs ROTATED by core*1024 rows and computes
the partial (sum loss_i, n_valid) over rotated rows 0..1023 (its anchor shard).
Pure SPMD; scalar partials summed on the host.

Key structure per core (vs the bf16 baseline):
  - the key side of the similarity is RAW: fT holds fp8(x) transposed
    straight from a stride-2 bf16 view of the fp32 rows (high bytes), so no
    per-key normalize multiply exists; the key-side 1/norm rides in the exp
    activation's per-partition scale AP (rinv10[:, kb]).  Only the 8 anchor
    chunks get explicitly normalized fp8 columns (fT_anch).
  - all big matmuls (similarity slab, class-sum accumulation, positives
    matmul) are fp8e4m3 with MatmulPerfMode.DoubleRow ([128, 2, *] operands,
    K=256 per pass): ~2x PE column rate on hardware.
  - 10/norm computed as exp(-0.5*ln(0.01*n2)) so every ACT call lives in the
    single natural_log_exp_and_others table -> no 1.3us activation-table
    reloads interleaved with the hot exp stream (Sqrt would thrash it).
  - the diagonal (self-pair) is removed by accumulating an extra -64*I fp8
    matmul into the similarity PSUM for key blocks 0..7: the exp argument
    10*r_i*(n_i - 64) <= -23 underflows to zero, so fp8 E never overflows.
  - g (10x class feature sums) uses scaled-onehot weights (DVE-built bf16)
    against the raw bf16 view; its 257th moving column holds n/10 so the same
    accumulation yields class counts to ~0.2%.
  - gpsimd (Pool) only builds the small [128,32] onehots: measured Pool cost
    is ~5x the cost model (generic tensor_scalar [128,256] is ~4us!), so no
    sizable tensor op may live there.
Engine budget (HW-measured): ACT exp stream ~77us but ~90% hidden; the wall
is the DVE chain (sq+evict+ohs+finale) + head/tail, ~150-185us total.
"""

import numpy as np

import bass_rust
import concourse.bass as bass
import concourse.tile as tile
from concourse import mybir
from concourse.bass_utils import run_bass_kernel_spmd

F32 = mybir.dt.float32
BF16 = mybir.dt.bfloat16
FP8 = mybir.dt.float8e4

B = 8192
H = 256
C = 32
N_CORES = 8
SHARD = B // N_CORES          # 1024 anchors per core
N_CHUNKS = B // 128           # 64 row chunks / key blocks
TEMP_INV = 10.0               # 1 / temperature
DR = mybir.MatmulPerfMode.DoubleRow

import os
ABLATE = frozenset(os.environ.get("BASS_ABLATE", "").split(",")) - {""}


class SplitDrainTileContext(tile.TileContext):
    """TileContext that caps sem waits at one per instruction.

    The walrus build in this container rejects instructions carrying more
    than one sync wait ("Too many sync wait commands", e.g. on Drain and
    TensorScalarPtr). Tile freely attaches several waits per instruction, so
    split the surplus onto same-engine nops inserted immediately before the
    instruction (identical semantics: the engine blocks on every wait before
    executing it).
    """

    MAX_DRAIN_WAITS = 1

    def _lower_ordered_insts(self, ordered):
        for insts in ordered.values():
            new_list = []
            for inst in insts:
                si = inst.sync_info
                ws = list(si.on_wait) if si is not None and si.on_wait else []
                if len(ws) > 1:
                    for k, w in enumerate(ws[:-1]):
                        new_list.append(mybir.InstNoOp(
                            name=f"{inst.name}_sw{k}",
                            engine=inst.engine,
                            bass_nofuse=True,
                            sync_info=mybir.SyncInfo(on_wait=[w], on_update=[]),
                        ))
                    inst.sync_info = mybir.SyncInfo(
                        on_wait=[ws[-1]], on_update=list(si.on_update or []))
                new_list.append(inst)
            insts[:] = new_list
        super()._lower_ordered_insts(ordered)

    def _drain_and_barrier(self, tick_clock, wait_clock):
        probe = self.nc.sync.nop()
        wait_clock.add_sem_waits(
            probe.ins, bass_rust.ScopedClock({None: tick_clock.global_clock})
        )
        si = probe.ins.sync_info
        waits = list(si.on_wait) if si is not None and si.on_wait else []
        probe.ins.sync_info = bass_rust.SyncInfo(
            on_wait=waits[: self.MAX_DRAIN_WAITS], on_update=[]
        )
        for i in range(self.MAX_DRAIN_WAITS, len(waits), self.MAX_DRAIN_WAITS):
            n = self.nc.sync.nop()
            n.ins.sync_info = bass_rust.SyncInfo(
                on_wait=waits[i : i + self.MAX_DRAIN_WAITS], on_update=[]
            )
        self.nc.sync.drain()

        self.nc.all_engine_barrier()
        assert self.sems is not None
        popped = self.nc._tile_sem_poison_stack.pop()
        assert popped is self._sem_poison
        self.nc.clear_and_free_semaphores(list(self.sems.allocated().values()))
        self.nc.all_engine_barrier()


def build_program(n_iters: int = 1):
    """Emit the SPMD program. n_iters > 1 wraps the body in a hardware loop
    (identical recompute) for wall-clock timing runs."""
    nc = bass.Bass("TRN2", target_bir_lowering=False, debug=False,
                   num_devices=N_CORES)

    feat = nc.dram_tensor("feat", [B, H], F32, kind="ExternalInput")
    lab = nc.dram_tensor("lab", [128, N_CHUNKS], F32, kind="ExternalInput")
    out = nc.dram_tensor("partials", [1, 2], F32, kind="ExternalOutput")

    with SplitDrainTileContext(nc) as tc:
        if n_iters == 1:
            emit_body(nc, tc, feat, lab, out)
        else:
            hints = (mybir.EngineType.PE, mybir.EngineType.Activation,
                     mybir.EngineType.DVE, mybir.EngineType.SP,
                     mybir.EngineType.Pool)
            with tc.For_i(0, n_iters, 1, hint_engines=hints):
                emit_body(nc, tc, feat, lab, out)
    return nc


def emit_body(nc, tc, feat, lab, out):
    from contextlib import ExitStack

    ACT = mybir.ActivationFunctionType
    OP = mybir.AluOpType
    AX = mybir.AxisListType

    with ExitStack() as ctx:
        ep = ctx.enter_context  # shorthand

        # ---- persistent SBUF ----
        const_pool = ep(tc.tile_pool(name="consts", bufs=1))
        id8 = const_pool.tile([128, 128], FP8)
        from concourse import masks
        masks.make_identity(nc, id8[:])
        id16 = const_pool.tile([128, 128], BF16)
        masks.make_identity(nc, id16[:])
        # dplate[:, q, :]: [128, 512] zeros except -64*I at column offset q*128
        # (keys are RAW rows: diag of ps is x_i . f_i_hat ~ n_i ~ 16; after -64
        # the exp argument 10*r_i*(n_i - 64) <= -23 underflows to zero)
        dplate = const_pool.tile([128, 4, 512], FP8)
        nc.gpsimd.memset(dplate[:], 0.0)
        for q in range(4):
            nc.gpsimd.affine_select(
                out=dplate[:, q, q * 128:(q + 1) * 128],
                in_=dplate[:, q, q * 128:(q + 1) * 128],
                compare_op=OP.not_equal,
                fill=-64.0, base=0, pattern=[[-1, 128]], channel_multiplier=1)
        iota32 = const_pool.tile([128, C], F32)
        nc.gpsimd.iota(iota32[:], pattern=[[1, C]], base=0,
                       channel_multiplier=0,
                       allow_small_or_imprecise_dtypes=True)
        ones32 = const_pool.tile([C, 1], F32)
        nc.gpsimd.memset(ones32[:], 1.0)
        ones16 = const_pool.tile([C, 1], BF16)
        nc.gpsimd.memset(ones16[:], 1.0)

        big_pool = ep(tc.tile_pool(name="big", bufs=1))
        # column H holds n_key/10 so the scaled-onehot g matmul also yields
        # exact-enough class counts: sum (10/n)*(n/10) = count
        x32 = big_pool.tile([128, N_CHUNKS, H + 1], F32)
        fT = big_pool.tile([128, 2, B], FP8)     # fT[p, k, j] = x[j, 128k+p] RAW
        fT_anch = big_pool.tile([128, 2, SHARD], FP8)  # normalized anchor cols
        oh_sb = big_pool.tile([128, N_CHUNKS, C], FP8)
        ohs_all = big_pool.tile([128, N_CHUNKS, C], BF16)
        rinv10 = big_pool.tile([128, N_CHUNKS], F32)   # 10 / ||x_key||
        lab_sb = big_pool.tile([128, N_CHUNKS], F32)
        nc.sync.dma_start(lab_sb[:], lab.ap())

        if ABLATE & {"nosq", "noevict"}:
            nc.gpsimd.memset(rinv10[:], 0.05)
            nc.gpsimd.memset(fT[:], 0.03)
            nc.gpsimd.memset(fT_anch[:], 0.03)
        # truncated-bf16 view of the high bytes of x32 (per chunk, free 256)
        x16v = x32[:].bitcast(mybir.dt.uint16).rearrange(
            "p c (h two) -> p c h two", two=2)[:, :, :, 1:2].bitcast(BF16)

        fr = feat.ap().rearrange("(c p) h -> p c h", p=128)
        for d in range(16):
            nc.sync.dma_start(x32[:, d * 4:(d + 1) * 4, 0:H],
                              fr[:, d * 4:(d + 1) * 4, :])

        # persistent PSUM accumulator for class sums
        cs_pool = ep(tc.tile_pool(name="csacc", bufs=1, space="PSUM"))
        cs_psum = cs_pool.tile([C, SHARD], F32)  # class_sum.T for anchors
        if "noexp" in ABLATE:
            for nb in range(2):
                nc.tensor.matmul(cs_psum[:, nb * 512:(nb + 1) * 512],
                                 id8[:, 0:C], id8[:, 0:512] if False else
                                 dplate[:, 0, :], start=True, stop=True)

        fin = ep(tc.tile_pool(name="fin", bufs=1))

        GRP = 8   # chunks per batched-rsqrt group
        LAG = 7   # hot-loop key block emitted alongside stage-A chunk kb+LAG

        cs_ready = []   # (pair_idx, e2 tile) with both exps emitted
        e2_cell = [None]

        def cs_mms(t, e2):
            for nb in range(2):
                nc.tensor.matmul(
                    cs_psum[:, nb * 512:(nb + 1) * 512],
                    oh_sb[:, 2 * t:2 * t + 2, :],
                    e2[:, :, nb * 512:(nb + 1) * 512],
                    start=(t == 0), stop=(t == N_CHUNKS // 2 - 1),
                    perf_mode=DR)

        with tc.tile_pool(name="simp", bufs=2, space="PSUM") as simpool, \
             tc.tile_pool(name="esb", bufs=3) as epool:

            def hot_iter(kb):
                ps = simpool.tile([128, SHARD], F32, tag="ps")
                for nb in range(2):
                    diag_here = (kb < SHARD // 128) and (kb * 128) // 512 == nb
                    nc.tensor.matmul(
                        ps[:, nb * 512:(nb + 1) * 512],
                        fT[:, :, kb * 128:(kb + 1) * 128],
                        fT_anch[:, :, nb * 512:(nb + 1) * 512],
                        start=True, stop=(not diag_here),
                        perf_mode=DR)
                    if diag_here:
                        # self-pairs: exp argument drops below -23 -> 0
                        nc.tensor.matmul(
                            ps[:, nb * 512:(nb + 1) * 512], id8[:],
                            dplate[:, kb % 4, :],
                            start=False, stop=True)
                if "noexp" in ABLATE:
                    return
                if cs_ready and kb % 2 == 0:
                    cs_mms(*cs_ready.pop(0))
                if kb % 2 == 0:
                    e2 = epool.tile([128, 2, SHARD], FP8, tag="e")
                    e2_cell[0] = e2
                e2 = e2_cell[0]
                nc.scalar.activation(e2[:, kb % 2, :], ps[:], ACT.Exp,
                                     scale=rinv10[:, kb:kb + 1])
                if kb % 2 == 1:
                    cs_ready.append((kb // 2, e2))

            # ---- stage A interleaved with the first hot-loop blocks ----
            with tc.tile_pool(name="gacc", bufs=1, space="PSUM") as g_pool, \
                 tc.tile_pool(name="transp", bufs=1, space="PSUM") as tp_pool, \
                 tc.tile_pool(name="sq", bufs=2) as sqpool, \
                 tc.tile_pool(name="ohs", bufs=2) as ohspool, \
                 tc.tile_pool(name="xba", bufs=2) as xbapool, \
                 tc.tile_pool(name="nrm", bufs=2) as npool:
                g_psum = g_pool.tile([C, H + 1], F32)  # 10*class feat sums|cnt

                def do_chunk(cj):
                    # scaled onehot (bf16): (iota==lab) * (10/n_key); the fp8
                    # onehot for the class-sum matmuls derives from it in
                    # PAIRS, keeping the slow Pool engine out of the chain
                    nc.vector.tensor_scalar(
                        out=ohs_all[:, cj, :], in0=iota32[:],
                        scalar1=lab_sb[:, cj:cj + 1],
                        scalar2=rinv10[:, cj:cj + 1],
                        op0=OP.is_equal, op1=OP.mult)
                    if cj % 2 == 1:
                        nc.vector.tensor_single_scalar(
                            oh_sb[:, cj - 1:cj + 1, :],
                            ohs_all[:, cj - 1:cj + 1, :], 0.25, OP.is_gt)
                    nc.tensor.matmul(
                        g_psum[:], ohs_all[:, cj, :],
                        x16v[:, cj:cj + 1, :, :],
                        start=(cj == 0), stop=(cj == N_CHUNKS - 1))
                    if "noevict" not in ABLATE:
                        tpu = tp_pool.tile([128, 512], mybir.dt.uint8,
                                           tag="tp")
                        tp16 = tpu[:].bitcast(BF16)
                        for kc in range(2):
                            nc.tensor.transpose(
                                tp16[:, kc * 128:(kc + 1) * 128],
                                x16v[:, cj:cj + 1,
                                     kc * 128:(kc + 1) * 128, :],
                                id16[:])
                        nc.vector.tensor_copy(
                            fT[:, :, cj * 128:(cj + 1) * 128],
                            tp16.rearrange("p (k n) -> p k n", k=2))
                        if cj < SHARD // 128:
                            # normalized anchor columns (chunks 0..7)
                            xba = xbapool.tile([128, H], FP8, tag="xba")
                            nc.vector.tensor_scalar(
                                out=xba[:], in0=x32[:, cj, 0:H],
                                scalar1=rinv10[:, cj:cj + 1], scalar2=0.1,
                                op0=OP.mult, op1=OP.mult)
                            tpu2 = tp_pool.tile([128, 512],
                                                mybir.dt.uint8, tag="tp")
                            tpa = tpu2[:].bitcast(FP8).rearrange(
                                "p (k n two) -> p k n two", k=2, two=2)
                            for kc in range(2):
                                nc.tensor.transpose(
                                    tpa[:, kc:kc + 1, :, 0:1],
                                    xba[:, kc * 128:(kc + 1) * 128],
                                    id8[:])
                            nc.vector.tensor_copy(
                                fT_anch[:, :, cj * 128:(cj + 1) * 128],
                                tpa[:, :, :, 0:1])
                    if cj == LAG:
                        hot_iter(0)
                        hot_iter(1)
                    elif LAG < cj < 2 * LAG:
                        hot_iter(2 * (cj - LAG))
                        hot_iter(2 * (cj - LAG) + 1)
                    elif cj >= 2 * LAG:
                        hot_iter(cj)

                for ch in range(N_CHUNKS):
                    g = ch % GRP
                    if g == 0:
                        n2g = npool.tile([128, GRP], F32, tag="n2")
                    sq = sqpool.tile([128, H], BF16, tag="sq")
                    if "nosq" in ABLATE and g == 0:
                        nc.vector.memset(n2g[:], 256.0)
                    if "nosq" not in ABLATE:
                        # n2 from the truncated-bf16 view (consistent with the
                        # fp8 fT quantization); bf16 in/out halves DVE traffic
                        nc.vector.scalar_tensor_tensor(
                            out=sq[:], in0=x16v[:, ch:ch + 1, 0:H, :],
                            scalar=0.0,
                            in1=x16v[:, ch:ch + 1, 0:H, :],
                            op0=OP.bypass, op1=OP.mult,
                            accum_out=n2g[:, g:g + 1])
                    if ch < GRP:
                        # group 0: per-chunk rsqrt so the anchor-column chain
                        # (gating the first hot block) starts ASAP
                        l2c = npool.tile([128, 1], F32, tag="l2c")
                        nc.scalar.activation(l2c[:], n2g[:, g:g + 1], ACT.Ln,
                                             scale=0.01)
                        nc.scalar.activation(rinv10[:, ch:ch + 1], l2c[:],
                                             ACT.Exp, scale=-0.5)
                        nc.vector.scalar_tensor_tensor(
                            out=x32[:, ch:ch + 1, H:H + 1],
                            in0=n2g[:, g:g + 1], scalar=0.01,
                            in1=rinv10[:, ch:ch + 1],
                            op0=OP.mult, op1=OP.mult)
                        do_chunk(ch)
                    elif g == GRP - 1:
                        # 10/norm = exp(-0.5*ln(n2) + ln 10): Exp/Ln table only
                        l2 = npool.tile([128, GRP], F32, tag="l2")
                        nc.scalar.activation(l2[:], n2g[:], ACT.Ln, scale=0.01)
                        nc.scalar.activation(
                            rinv10[:, ch - (GRP - 1):ch + 1], l2[:], ACT.Exp,
                            scale=-0.5)
                        # n/10 = n2 * (10/n) / 100 into column H
                        nc.vector.scalar_tensor_tensor(
                            out=x32[:, ch - (GRP - 1):ch + 1, H:H + 1],
                            in0=n2g[:], scalar=0.01,
                            in1=rinv10[:, ch - (GRP - 1):ch + 1],
                            op0=OP.mult, op1=OP.mult)
                        for j in range(GRP):
                            do_chunk(ch - (GRP - 1) + j)

                # ---- pre-tail finale work (independent of class sums) ----
                g8 = fin.tile([C, H], FP8)
                nc.vector.tensor_copy(g8[:], g_psum[:, 0:H])
                counts = fin.tile([C, 1], F32)
                nc.vector.tensor_copy(counts[:], g_psum[:, H:H + 1])

                gT8 = fin.tile([128, 2, C], FP8)
                for kc in range(2):
                    tpgu = tp_pool.tile([128, 512], mybir.dt.uint8, tag="tp")
                    tpg = tpgu[:].bitcast(FP8).rearrange(
                        "p (k n two) -> p k n two", k=2, two=2)
                    nc.tensor.transpose(
                        tpg[:, 0:1, 0:C, 0:1],
                        g8[:, kc * 128:(kc + 1) * 128], id8[0:C, 0:C])
                    nc.vector.tensor_copy(gT8[:, kc:kc + 1, :],
                                          tpg[:, 0:1, 0:C, 0:1])
                ohT = fin.tile([C, SHARD], BF16)
                for bkl in range(SHARD // 128):
                    tpou = tp_pool.tile([128, 512], mybir.dt.uint8, tag="tp")
                    tpo = tpou[:].bitcast(FP8).rearrange(
                        "p (k n two) -> p k n two", k=2, two=2)
                    nc.tensor.transpose(
                        tpo[0:C, 0:1, :, 0:1], oh_sb[:, bkl:bkl + 1, :],
                        id8[:])
                    nc.vector.tensor_copy(
                        ohT[:, bkl * 128:(bkl + 1) * 128],
                        tpo[0:C, 0:1, :, 0:1])

            with tc.tile_pool(name="rpsum", bufs=1, space="PSUM") as rpool:
                r_psum = rpool.tile([C, SHARD], F32)
                for nb in range(2):
                    nc.tensor.matmul(
                        r_psum[:, nb * 512:(nb + 1) * 512], gT8[:],
                        fT_anch[:, :, nb * 512:(nb + 1) * 512],
                        start=True, stop=True, perf_mode=DR)

                # counts_excl, masks, positives numerator / denominator
                ce = fin.tile([C, SHARD], BF16)
                nc.vector.tensor_scalar(
                    out=ce[:], in0=ohT[:], scalar1=counts[:], scalar2=-1.0,
                    op0=OP.subtract, op1=OP.mult)
                mask = fin.tile([C, SHARD], BF16)
                nc.vector.tensor_single_scalar(mask[:], ce[:], 0.5, OP.is_gt)
                ce1 = fin.tile([C, SHARD], BF16)
                nc.vector.tensor_single_scalar(ce1[:], ce[:], 1.0, OP.max)
                rce = fin.tile([C, SHARD], BF16)
                with nc.allow_low_precision(
                        reason="1/count at bf16: 0.4% on denom terms, "
                               "well under the 2e-2 gate"):
                    nc.vector.reciprocal(rce[:], ce1[:])
                nc.vector.tensor_tensor(rce[:], rce[:], mask[:], OP.mult)

                # r_psum rows hold 10*sum_{j in c} cos(i,j); self term ~ 10
                pnum = fin.tile([C, SHARD], BF16)
                nc.vector.scalar_tensor_tensor(
                    out=pnum[:], in0=r_psum[:], scalar=10.0, in1=ohT[:],
                    op0=OP.subtract, op1=OP.mult)
                pden = fin.tile([C, SHARD], BF16)
                nc.vector.tensor_tensor(pden[:], ohT[:], ce[:], OP.mult)

            with tc.tile_pool(name="rows_pre", bufs=1, space="PSUM") as rowp:
                prow = fin.tile([1, SHARD], F32)
                posrow = fin.tile([1, SHARD], F32)
                for src, dst in ((pden, prow), (pnum, posrow)):
                    row = rowp.tile([1, SHARD], F32, tag="row")
                    for nb in range(2):
                        nc.tensor.matmul(
                            row[:, nb * 512:(nb + 1) * 512], ones16[:],
                            src[:, nb * 512:(nb + 1) * 512],
                            start=True, stop=True)
                    nc.vector.tensor_copy(dst[:], row[:])

                valid = fin.tile([1, SHARD], F32)
                nc.vector.tensor_single_scalar(
                    valid[:], prow[:], 0.5, OP.is_gt)
                nc.vector.tensor_single_scalar(prow[:], prow[:], 1.0, OP.max)
                rp = fin.tile([1, SHARD], F32)
                nc.vector.reciprocal(rp[:], prow[:])
                mp = fin.tile([1, SHARD], F32)
                nc.vector.scalar_tensor_tensor(
                    out=mp[:], in0=posrow[:], scalar=1.0, in1=rp[:],
                    op0=OP.mult, op1=OP.mult)

            # all hot blocks were emitted inside stage A; flush class sums
            while cs_ready:
                cs_mms(*cs_ready.pop(0))

        # ---- post-hot finale: denominator path and output ----
        with tc.tile_pool(name="rows_post", bufs=1, space="PSUM") as rowpool:
            terms = fin.tile([C, SHARD], BF16)
            nc.vector.tensor_tensor(terms[:], cs_psum[:], rce[:], OP.mult)
            logd = fin.tile([1, SHARD], F32)
            row = rowpool.tile([1, SHARD], F32)
            for nb in range(2):
                nc.tensor.matmul(
                    row[:, nb * 512:(nb + 1) * 512], ones16[:],
                    terms[:, nb * 512:(nb + 1) * 512],
                    start=True, stop=True)
            nc.vector.tensor_single_scalar(logd[:], row[:], 1e-30, OP.max)
            nc.scalar.activation(logd[:], logd[:], ACT.Ln)

            li = fin.tile([1, SHARD], F32)
            nc.vector.tensor_tensor(li[:], logd[:], mp[:], OP.subtract)
            nc.vector.tensor_tensor(li[:], li[:], valid[:], OP.mult)

            res = fin.tile([1, 2], F32)
            nc.vector.tensor_reduce(res[:, 0:1], li[:], axis=AX.X, op=OP.add)
            nc.vector.tensor_reduce(res[:, 1:2], valid[:], axis=AX.X, op=OP.add)
            nc.sync.dma_start(out.ap(), res[:])


_PROGRAM_CACHE = {}


def get_program(n_iters: int = 1):
    if n_iters not in _PROGRAM_CACHE:
        _PROGRAM_CACHE[n_iters] = build_program(n_iters)
    return _PROGRAM_CACHE[n_iters]


def make_in_maps(features: np.ndarray, labels: np.ndarray):
    features = np.ascontiguousarray(np.asarray(features, dtype=np.float32))
    labels_f = np.asarray(labels).astype(np.float32)
    in_maps = []
    for c in range(N_CORES):
        fr = np.roll(features, -c * SHARD, axis=0)
        lr = np.roll(labels_f, -c * SHARD)
        in_maps.append({
            "feat": np.ascontiguousarray(fr),
            "lab": np.ascontiguousarray(lr.reshape(N_CHUNKS, 128).T),
        })
    return in_maps


def kernel(features, labels):
    nc = get_program(1)
    in_maps = make_in_maps(features, labels)
    res = run_bass_kernel_spmd(nc, in_maps, list(range(N_CORES)))
    loss_sum = np.float32(0.0)
    n_valid = np.float32(0.0)
    for c in range(N_CORES):
        p = res.results[c]["partials"]
        loss_sum += np.float32(p[0, 0])
        n_valid += np.float32(p[0, 1])
    if n_valid > 0:
        loss = loss_sum / np.float32(max(n_valid, 1.0))
    else:
        loss = np.float32(0.0)
    return np.array(loss, dtype=np.float32)


# revision 40
# speedup vs baseline: 1.2576x; 1.0032x over previous
"""BSC loss (single label) on 8 Trainium2 NeuronCores — fp8 DoubleRow version.

Reference computation (B=8192, H=256, C=32, T=0.1):
    f   = l2_normalize(features)                      # [B, H]
    sim = f @ f.T / T                                 # [B, B] (never materialized)
    E   = exp(sim) with zeroed diagonal
    class_sum[i, c] = sum_{j: label_j = c} E[i, j]
    counts_excl[i, c] = counts[c] - onehot[i, c]
    denom_i = sum_c where(ce > 0, class_sum / max(ce, 1))
    mean_pos_sim_i = (sum_{j != i, same label} sim[i, j]) / P_i
    loss_i = log(max(denom_i, 1e-30)) - mean_pos_sim_i   (if P_i > 0)
    loss = sum(loss_i) / n_valid

Distribution: each core gets the inputs ROTATED by core*1024 rows and computes
the partial (sum loss_i, n_valid) over rotated rows 0..1023 (its anchor shard).
Pure SPMD; scalar partials summed on the host.

Key structure per core (vs the bf16 baseline):
  - the key side of the similarity is RAW: fT holds fp8(x) transposed
    straight from a stride-2 bf16 view of the fp32 rows (high bytes), so no
    per-key normalize multiply exists; the key-side 1/norm rides in the exp
    activation's per-partition scale AP (rinv10[:, kb]).  Only the 8 anchor
    chunks get explicitly normalized fp8 columns (fT_anch).
  - all big matmuls (similarity slab, class-sum accumulation, positives
    matmul) are fp8e4m3 with MatmulPerfMode.DoubleRow ([128, 2, *] operands,
    K=256 per pass): ~2x PE column rate on hardware.
  - 10/norm computed as exp(-0.5*ln(0.01*n2)) so every ACT call lives in the
    single natural_log_exp_and_others table -> no 1.3us activation-table
    reloads interleaved with the hot exp stream (Sqrt would thrash it).
  - the diagonal (self-pair) is removed by accumulating an extra -64*I fp8
    matmul into the similarity PSUM for key blocks 0..7: the exp argument
    10*r_i*(n_i - 64) <= -23 underflows to zero, so fp8 E never overflows.
  - g (10x class feature sums) uses scaled-onehot weights (DVE-built bf16)
    against the raw bf16 view; its 257th moving column holds n/10 so the same
    accumulation yields class counts to ~0.2%.
  - the Pool/gpsimd engine is OFF the per-chunk chain entirely (one-time
    constant setup only): measured Pool per-op cost is 5-10x the cost model
    (762ns for a [128,32] tensor_scalar, ~4us for [128,256]).  The scaled
    onehot persists for all chunks and the fp8 onehot derives from it in
    pairs on DVE (~90ns/chunk amortized).
Engine budget (HW-measured): ACT exp stream ~77us, almost fully hidden; the
wall is the DVE chain (sq+evict+ohs) + PE + head/tail, ~140us total.
"""

import numpy as np

import bass_rust
import concourse.bass as bass
import concourse.tile as tile
from concourse import mybir
from concourse.bass_utils import run_bass_kernel_spmd

F32 = mybir.dt.float32
BF16 = mybir.dt.bfloat16
FP8 = mybir.dt.float8e4

B = 8192
H = 256
C = 32
N_CORES = 8
SHARD = B // N_CORES          # 1024 anchors per core
N_CHUNKS = B // 128           # 64 row chunks / key blocks
TEMP_INV = 10.0               # 1 / temperature
DR = mybir.MatmulPerfMode.DoubleRow

import os
ABLATE = frozenset(os.environ.get("BASS_ABLATE", "").split(",")) - {""}


class SplitDrainTileContext(tile.TileContext):
    """TileContext that caps sem waits at one per instruction.

    The walrus build in this container rejects instructions carrying more
    than one sync wait ("Too many sync wait commands", e.g. on Drain and
    TensorScalarPtr). Tile freely attaches several waits per instruction, so
    split the surplus onto same-engine nops inserted immediately before the
    instruction (identical semantics: the engine blocks on every wait before
    executing it).
    """

    MAX_DRAIN_WAITS = 1

    def _lower_ordered_insts(self, ordered):
        for insts in ordered.values():
            new_list = []
            for inst in insts:
                si = inst.sync_info
                ws = list(si.on_wait) if si is not None and si.on_wait else []
                if len(ws) > 1:
                    for k, w in enumerate(ws[:-1]):
                        new_list.append(mybir.InstNoOp(
                            name=f"{inst.name}_sw{k}",
                            engine=inst.engine,
                            bass_nofuse=True,
                            sync_info=mybir.SyncInfo(on_wait=[w], on_update=[]),
                        ))
                    inst.sync_info = mybir.SyncInfo(
                        on_wait=[ws[-1]], on_update=list(si.on_update or []))
                new_list.append(inst)
            insts[:] = new_list
        super()._lower_ordered_insts(ordered)

    def _drain_and_barrier(self, tick_clock, wait_clock):
        probe = self.nc.sync.nop()
        wait_clock.add_sem_waits(
            probe.ins, bass_rust.ScopedClock({None: tick_clock.global_clock})
        )
        si = probe.ins.sync_info
        waits = list(si.on_wait) if si is not None and si.on_wait else []
        probe.ins.sync_info = bass_rust.SyncInfo(
            on_wait=waits[: self.MAX_DRAIN_WAITS], on_update=[]
        )
        for i in range(self.MAX_DRAIN_WAITS, len(waits), self.MAX_DRAIN_WAITS):
            n = self.nc.sync.nop()
            n.ins.sync_info = bass_rust.SyncInfo(
                on_wait=waits[i : i + self.MAX_DRAIN_WAITS], on_update=[]
            )
        self.nc.sync.drain()

        self.nc.all_engine_barrier()
        assert self.sems is not None
        popped = self.nc._tile_sem_poison_stack.pop()
        assert popped is self._sem_poison
        self.nc.clear_and_free_semaphores(list(self.sems.allocated().values()))
        self.nc.all_engine_barrier()


def build_program(n_iters: int = 1):
    """Emit the SPMD program. n_iters > 1 wraps the body in a hardware loop
    (identical recompute) for wall-clock timing runs."""
    nc = bass.Bass("TRN2", target_bir_lowering=False, debug=False,
                   num_devices=N_CORES)

    feat = nc.dram_tensor("feat", [B, H], F32, kind="ExternalInput")
    lab = nc.dram_tensor("lab", [128, N_CHUNKS], F32, kind="ExternalInput")
    out = nc.dram_tensor("partials", [1, 2], F32, kind="ExternalOutput")

    with SplitDrainTileContext(nc) as tc:
        if n_iters == 1:
            emit_body(nc, tc, feat, lab, out)
        else:
            hints = (mybir.EngineType.PE, mybir.EngineType.Activation,
                     mybir.EngineType.DVE, mybir.EngineType.SP,
                     mybir.EngineType.Pool)
            with tc.For_i(0, n_iters, 1, hint_engines=hints):
                emit_body(nc, tc, feat, lab, out)
    return nc


def emit_body(nc, tc, feat, lab, out):
    from contextlib import ExitStack

    ACT = mybir.ActivationFunctionType
    OP = mybir.AluOpType
    AX = mybir.AxisListType

    with ExitStack() as ctx:
        ep = ctx.enter_context  # shorthand

        # ---- persistent SBUF ----
        const_pool = ep(tc.tile_pool(name="consts", bufs=1))
        id8 = const_pool.tile([128, 128], FP8)
        from concourse import masks
        masks.make_identity(nc, id8[:])
        id16 = const_pool.tile([128, 128], BF16)
        masks.make_identity(nc, id16[:])
        # dplate[:, q, :]: [128, 512] zeros except -64*I at column offset q*128
        # (keys are RAW rows: diag of ps is x_i . f_i_hat ~ n_i ~ 16; after -64
        # the exp argument 10*r_i*(n_i - 64) <= -23 underflows to zero)
        dplate = const_pool.tile([128, 4, 512], FP8)
        nc.gpsimd.memset(dplate[:], 0.0)
        for q in range(4):
            nc.gpsimd.affine_select(
                out=dplate[:, q, q * 128:(q + 1) * 128],
                in_=dplate[:, q, q * 128:(q + 1) * 128],
                compare_op=OP.not_equal,
                fill=-64.0, base=0, pattern=[[-1, 128]], channel_multiplier=1)
        iota32 = const_pool.tile([128, C], F32)
        nc.gpsimd.iota(iota32[:], pattern=[[1, C]], base=0,
                       channel_multiplier=0,
                       allow_small_or_imprecise_dtypes=True)
        ones32 = const_pool.tile([C, 1], F32)
        nc.gpsimd.memset(ones32[:], 1.0)
        ones16 = const_pool.tile([C, 1], BF16)
        nc.gpsimd.memset(ones16[:], 1.0)

        big_pool = ep(tc.tile_pool(name="big", bufs=1))
        # column H holds n_key/10 so the scaled-onehot g matmul also yields
        # exact-enough class counts: sum (10/n)*(n/10) = count
        x32 = big_pool.tile([128, N_CHUNKS, H + 1], F32)
        fT = big_pool.tile([128, 2, B], FP8)     # fT[p, k, j] = x[j, 128k+p] RAW
        fT_anch = big_pool.tile([128, 2, SHARD], FP8)  # normalized anchor cols
        oh_sb = big_pool.tile([128, N_CHUNKS, C], FP8)
        ohs_all = big_pool.tile([128, N_CHUNKS, C], BF16)
        rinv10 = big_pool.tile([128, N_CHUNKS], F32)   # 10 / ||x_key||
        lab_sb = big_pool.tile([128, N_CHUNKS], F32)
        nc.sync.dma_start(lab_sb[:], lab.ap())

        if ABLATE & {"nosq", "noevict"}:
            nc.gpsimd.memset(rinv10[:], 0.05)
            nc.gpsimd.memset(fT[:], 0.03)
            nc.gpsimd.memset(fT_anch[:], 0.03)
        # truncated-bf16 view of the high bytes of x32 (per chunk, free 256)
        x16v = x32[:].bitcast(mybir.dt.uint16).rearrange(
            "p c (h two) -> p c h two", two=2)[:, :, :, 1:2].bitcast(BF16)

        fr = feat.ap().rearrange("(c p) h -> p c h", p=128)
        for d in range(16):
            nc.sync.dma_start(x32[:, d * 4:(d + 1) * 4, 0:H],
                              fr[:, d * 4:(d + 1) * 4, :])

        # persistent PSUM accumulator for class sums
        cs_pool = ep(tc.tile_pool(name="csacc", bufs=1, space="PSUM"))
        cs_psum = cs_pool.tile([C, SHARD], F32)  # class_sum.T for anchors
        if "noexp" in ABLATE:
            for nb in range(2):
                nc.tensor.matmul(cs_psum[:, nb * 512:(nb + 1) * 512],
                                 id8[:, 0:C], id8[:, 0:512] if False else
                                 dplate[:, 0, :], start=True, stop=True)

        fin = ep(tc.tile_pool(name="fin", bufs=1))

        GRP = 8   # chunks per batched-rsqrt group
        LAG = 7   # hot-loop key block emitted alongside stage-A chunk kb+LAG

        cs_ready = []   # (pair_idx, e2 tile) with both exps emitted
        e2_cell = [None]

        def cs_mms(t, e2):
            for nb in range(2):
                nc.tensor.matmul(
                    cs_psum[:, nb * 512:(nb + 1) * 512],
                    oh_sb[:, 2 * t:2 * t + 2, :],
                    e2[:, :, nb * 512:(nb + 1) * 512],
                    start=(t == 0), stop=(t == N_CHUNKS // 2 - 1),
                    perf_mode=DR)

        with tc.tile_pool(name="simp", bufs=2, space="PSUM") as simpool, \
             tc.tile_pool(name="esb", bufs=3) as epool:

            def hot_iter(kb):
                ps = simpool.tile([128, SHARD], F32, tag="ps")
                for nb in range(2):
                    diag_here = (kb < SHARD // 128) and (kb * 128) // 512 == nb
                    nc.tensor.matmul(
                        ps[:, nb * 512:(nb + 1) * 512],
                        fT[:, :, kb * 128:(kb + 1) * 128],
                        fT_anch[:, :, nb * 512:(nb + 1) * 512],
                        start=True, stop=(not diag_here),
                        perf_mode=DR)
                    if diag_here:
                        # self-pairs: exp argument drops below -23 -> 0
                        nc.tensor.matmul(
                            ps[:, nb * 512:(nb + 1) * 512], id8[:],
                            dplate[:, kb % 4, :],
                            start=False, stop=True)
                if "noexp" in ABLATE:
                    return
                if cs_ready and kb % 2 == 0:
                    cs_mms(*cs_ready.pop(0))
                if kb % 2 == 0:
                    e2 = epool.tile([128, 2, SHARD], FP8, tag="e")
                    e2_cell[0] = e2
                e2 = e2_cell[0]
                nc.scalar.activation(e2[:, kb % 2, :], ps[:], ACT.Exp,
                                     scale=rinv10[:, kb:kb + 1])
                if kb % 2 == 1:
                    cs_ready.append((kb // 2, e2))

            # ---- stage A interleaved with the first hot-loop blocks ----
            with tc.tile_pool(name="gacc", bufs=1, space="PSUM") as g_pool, \
                 tc.tile_pool(name="transp", bufs=1, space="PSUM") as tp_pool, \
                 tc.tile_pool(name="sq", bufs=2) as sqpool, \
                 tc.tile_pool(name="ohs", bufs=2) as ohspool, \
                 tc.tile_pool(name="xba", bufs=2) as xbapool, \
                 tc.tile_pool(name="nrm", bufs=2) as npool:
                g_psum = g_pool.tile([C, H + 1], F32)  # 10*class feat sums|cnt

                def do_chunk(cj):
                    # scaled onehot (bf16): (iota==lab) * (10/n_key); the fp8
                    # onehot for the class-sum matmuls derives from it in
                    # PAIRS, keeping the slow Pool engine out of the chain
                    nc.vector.tensor_scalar(
                        out=ohs_all[:, cj, :], in0=iota32[:],
                        scalar1=lab_sb[:, cj:cj + 1],
                        scalar2=rinv10[:, cj:cj + 1],
                        op0=OP.is_equal, op1=OP.mult)
                    if cj % 2 == 1:
                        nc.vector.tensor_single_scalar(
                            oh_sb[:, cj - 1:cj + 1, :],
                            ohs_all[:, cj - 1:cj + 1, :], 0.25, OP.is_gt)
                    nc.tensor.matmul(
                        g_psum[:], ohs_all[:, cj, :],
                        x16v[:, cj:cj + 1, :, :],
                        start=(cj == 0), stop=(cj == N_CHUNKS - 1))
                    if "noevict" not in ABLATE:
                        tpu = tp_pool.tile([128, 512], mybir.dt.uint8,
                                           tag="tp")
                        tp16 = tpu[:].bitcast(BF16)
                        for kc in range(2):
                            nc.tensor.transpose(
                                tp16[:, kc * 128:(kc + 1) * 128],
                                x16v[:, cj:cj + 1,
                                     kc * 128:(kc + 1) * 128, :],
                                id16[:])
                        nc.vector.tensor_copy(
                            fT[:, :, cj * 128:(cj + 1) * 128],
                            tp16.rearrange("p (k n) -> p k n", k=2))
                        if cj < SHARD // 128:
                            # normalized anchor columns (chunks 0..7)
                            xba = xbapool.tile([128, H], FP8, tag="xba")
                            nc.vector.tensor_scalar(
                                out=xba[:], in0=x32[:, cj, 0:H],
                                scalar1=rinv10[:, cj:cj + 1], scalar2=0.1,
                                op0=OP.mult, op1=OP.mult)
                            tpu2 = tp_pool.tile([128, 512],
                                                mybir.dt.uint8, tag="tp")
                            tpa = tpu2[:].bitcast(FP8).rearrange(
                                "p (k n two) -> p k n two", k=2, two=2)
                            for kc in range(2):
                                nc.tensor.transpose(
                                    tpa[:, kc:kc + 1, :, 0:1],
                                    xba[:, kc * 128:(kc + 1) * 128],
                                    id8[:])
                            nc.vector.tensor_copy(
                                fT_anch[:, :, cj * 128:(cj + 1) * 128],
                                tpa[:, :, :, 0:1])
                    if cj == LAG:
                        hot_iter(0)
                        hot_iter(1)
                    elif LAG < cj < 2 * LAG:
                        hot_iter(2 * (cj - LAG))
                        hot_iter(2 * (cj - LAG) + 1)
                    elif cj >= 2 * LAG:
                        hot_iter(cj)

                for ch in range(N_CHUNKS):
                    g = ch % GRP
                    if g == 0:
                        n2g = npool.tile([128, GRP], F32, tag="n2")
                    sq = sqpool.tile([128, H], BF16, tag="sq")
                    if "nosq" in ABLATE and g == 0:
                        nc.vector.memset(n2g[:], 256.0)
                    if "nosq" not in ABLATE:
                        # n2 from the truncated-bf16 view (consistent with the
                        # fp8 fT quantization); bf16 in/out halves DVE traffic
                        nc.vector.scalar_tensor_tensor(
                            out=sq[:], in0=x16v[:, ch:ch + 1, 0:H, :],
                            scalar=0.0,
                            in1=x16v[:, ch:ch + 1, 0:H, :],
                            op0=OP.bypass, op1=OP.mult,
                            accum_out=n2g[:, g:g + 1])
                    if ch < GRP:
                        # group 0: per-chunk rsqrt so the anchor-column chain
                        # (gating the first hot block) starts ASAP
                        l2c = npool.tile([128, 1], F32, tag="l2c")
                        nc.scalar.activation(l2c[:], n2g[:, g:g + 1], ACT.Ln,
                                             scale=0.01)
                        nc.scalar.activation(rinv10[:, ch:ch + 1], l2c[:],
                                             ACT.Exp, scale=-0.5)
                        nc.vector.scalar_tensor_tensor(
                            out=x32[:, ch:ch + 1, H:H + 1],
                            in0=n2g[:, g:g + 1], scalar=0.01,
                            in1=rinv10[:, ch:ch + 1],
                            op0=OP.mult, op1=OP.mult)
                        do_chunk(ch)
                    elif g == GRP - 1:
                        # 10/norm = exp(-0.5*ln(n2) + ln 10): Exp/Ln table only
                        l2 = npool.tile([128, GRP], F32, tag="l2")
                        nc.scalar.activation(l2[:], n2g[:], ACT.Ln, scale=0.01)
                        nc.scalar.activation(
                            rinv10[:, ch - (GRP - 1):ch + 1], l2[:], ACT.Exp,
                            scale=-0.5)
                        # n/10 = n2 * (10/n) / 100 into column H
                        nc.vector.scalar_tensor_tensor(
                            out=x32[:, ch - (GRP - 1):ch + 1, H:H + 1],
                            in0=n2g[:], scalar=0.01,
                            in1=rinv10[:, ch - (GRP - 1):ch + 1],
                            op0=OP.mult, op1=OP.mult)
                        for j in range(GRP):
                            do_chunk(ch - (GRP - 1) + j)

                # ---- pre-tail finale work (independent of class sums) ----
                g8 = fin.tile([C, H], FP8)
                nc.vector.tensor_copy(g8[:], g_psum[:, 0:H])
                counts = fin.tile([C, 1], F32)
                nc.vector.tensor_copy(counts[:], g_psum[:, H:H + 1])

                gT8 = fin.tile([128, 2, C], FP8)
                for kc in range(2):
                    tpgu = tp_pool.tile([128, 512], mybir.dt.uint8, tag="tp")
                    tpg = tpgu[:].bitcast(FP8).rearrange(
                        "p (k n two) -> p k n two", k=2, two=2)
                    nc.tensor.transpose(
                        tpg[:, 0:1, 0:C, 0:1],
                        g8[:, kc * 128:(kc + 1) * 128], id8[0:C, 0:C])
                    nc.vector.tensor_copy(gT8[:, kc:kc + 1, :],
                                          tpg[:, 0:1, 0:C, 0:1])
                ohT = fin.tile([C, SHARD], BF16)
                for bkl in range(SHARD // 128):
                    tpou = tp_pool.tile([128, 512], mybir.dt.uint8, tag="tp")
                    tpo = tpou[:].bitcast(FP8).rearrange(
                        "p (k n two) -> p k n two", k=2, two=2)
                    nc.tensor.transpose(
                        tpo[0:C, 0:1, :, 0:1], oh_sb[:, bkl:bkl + 1, :],
                        id8[:])
                    nc.vector.tensor_copy(
                        ohT[:, bkl * 128:(bkl + 1) * 128],
                        tpo[0:C, 0:1, :, 0:1])

            with tc.tile_pool(name="rpsum", bufs=1, space="PSUM") as rpool:
                r_psum = rpool.tile([C, SHARD], F32)
                for nb in range(2):
                    nc.tensor.matmul(
                        r_psum[:, nb * 512:(nb + 1) * 512], gT8[:],
                        fT_anch[:, :, nb * 512:(nb + 1) * 512],
                        start=True, stop=True, perf_mode=DR)

                # counts_excl, masks, positives numerator / denominator
                ce = fin.tile([C, SHARD], BF16)
                nc.vector.tensor_scalar(
                    out=ce[:], in0=ohT[:], scalar1=counts[:], scalar2=-1.0,
                    op0=OP.subtract, op1=OP.mult)
                mask = fin.tile([C, SHARD], BF16)
                nc.vector.tensor_single_scalar(mask[:], ce[:], 0.5, OP.is_gt)
                ce1 = fin.tile([C, SHARD], BF16)
                nc.vector.tensor_single_scalar(ce1[:], ce[:], 1.0, OP.max)
                rce = fin.tile([C, SHARD], BF16)
                with nc.allow_low_precision(
                        reason="1/count at bf16: 0.4% on denom terms, "
                               "well under the 2e-2 gate"):
                    nc.vector.reciprocal(rce[:], ce1[:])
                nc.vector.tensor_tensor(rce[:], rce[:], mask[:], OP.mult)

                # r_psum rows hold 10*sum_{j in c} cos(i,j); self term ~ 10
                pnum = fin.tile([C, SHARD], BF16)
                nc.vector.scalar_tensor_tensor(
                    out=pnum[:], in0=r_psum[:], scalar=10.0, in1=ohT[:],
                    op0=OP.subtract, op1=OP.mult)
                pden = fin.tile([C, SHARD], BF16)
                nc.vector.tensor_tensor(pden[:], ohT[:], ce[:], OP.mult)

            with tc.tile_pool(name="rows_pre", bufs=1, space="PSUM") as rowp:
                prow = fin.tile([1, SHARD], F32)
                posrow = fin.tile([1, SHARD], F32)
                for src, dst in ((pden, prow), (pnum, posrow)):
                    row = rowp.tile([1, SHARD], F32, tag="row")
                    for nb in range(2):
                        nc.tensor.matmul(
                            row[:, nb * 512:(nb + 1) * 512], ones16[:],
                            src[:, nb * 512:(nb + 1) * 512],
                            start=True, stop=True)
                    nc.vector.tensor_copy(dst[:], row[:])

                valid = fin.tile([1, SHARD], F32)
                nc.vector.tensor_single_scalar(
                    valid[:], prow[:], 0.5, OP.is_gt)
                nc.vector.tensor_single_scalar(prow[:], prow[:], 1.0, OP.max)
                rp = fin.tile([1, SHARD], F32)
                nc.vector.reciprocal(rp[:], prow[:])
                mp = fin.tile([1, SHARD], F32)
                nc.vector.scalar_tensor_tensor(
                    out=mp[:], in0=posrow[:], scalar=1.0, in1=rp[:],
                    op0=OP.mult, op1=OP.mult)

            # all hot blocks were emitted inside stage A; flush class sums
            while cs_ready:
                cs_mms(*cs_ready.pop(0))

        # ---- post-hot finale: denominator path and output ----
        with tc.tile_pool(name="rows_post", bufs=1, space="PSUM") as rowpool:
            terms = fin.tile([C, SHARD], BF16)
            nc.vector.tensor_tensor(terms[:], cs_psum[:], rce[:], OP.mult)
            logd = fin.tile([1, SHARD], F32)
            row = rowpool.tile([1, SHARD], F32)
            for nb in range(2):
                nc.tensor.matmul(
                    row[:, nb * 512:(nb + 1) * 512], ones16[:],
                    terms[:, nb * 512:(nb + 1) * 512],
                    start=True, stop=True)
            nc.vector.tensor_single_scalar(logd[:], row[:], 1e-30, OP.max)
            nc.scalar.activation(logd[:], logd[:], ACT.Ln)

            li = fin.tile([1, SHARD], F32)
            nc.vector.tensor_tensor(li[:], logd[:], mp[:], OP.subtract)
            nc.vector.tensor_tensor(li[:], li[:], valid[:], OP.mult)

            res = fin.tile([1, 2], F32)
            nc.vector.tensor_reduce(res[:, 0:1], li[:], axis=AX.X, op=OP.add)
            nc.vector.tensor_reduce(res[:, 1:2], valid[:], axis=AX.X, op=OP.add)
            nc.sync.dma_start(out.ap(), res[:])


_PROGRAM_CACHE = {}


def get_program(n_iters: int = 1):
    if n_iters not in _PROGRAM_CACHE:
        _PROGRAM_CACHE[n_iters] = build_program(n_iters)
    return _PROGRAM_CACHE[n_iters]


def make_in_maps(features: np.ndarray, labels: np.ndarray):
    features = np.ascontiguousarray(np.asarray(features, dtype=np.float32))
    labels_f = np.asarray(labels).astype(np.float32)
    in_maps = []
    for c in range(N_CORES):
        fr = np.roll(features, -c * SHARD, axis=0)
        lr = np.roll(labels_f, -c * SHARD)
        in_maps.append({
            "feat": np.ascontiguousarray(fr),
            "lab": np.ascontiguousarray(lr.reshape(N_CHUNKS, 128).T),
        })
    return in_maps


def kernel(features, labels):
    nc = get_program(1)
    in_maps = make_in_maps(features, labels)
    res = run_bass_kernel_spmd(nc, in_maps, list(range(N_CORES)))
    loss_sum = np.float32(0.0)
    n_valid = np.float32(0.0)
    for c in range(N_CORES):
        p = res.results[c]["partials"]
        loss_sum += np.float32(p[0, 0])
        n_valid += np.float32(p[0, 1])
    if n_valid > 0:
        loss = loss_sum / np.float32(max(n_valid, 1.0))
    else:
        loss = np.float32(0.0)
    return np.array(loss, dtype=np.float32)
